# revision 4
# baseline (speedup 1.0000x reference)
"""PolarToCartesianGrid scatter-add kernel for 8 Trainium2 NeuronCores.

Strategy (voxel-range sharded, all 16 batch samples as partition lanes):
  host: sort polar cells by target voxel (indices are compile-time data);
        cut the sorted stream into segment-aligned "pieces" (<=12288 cells,
        <=65536 voxel span); pack pieces into 64 slot-streams
        (8 cores x 8 partition-groups) balancing gather windows.
  device (per core):
        - DMA in values [128, 16384] (partition = 16*slot + sample lane)
        - tensor_tensor_scan(mult,add) with a reset mask => running segment
          sums; each voxel's total sits at its segment-end position (fp32)
        - ap_gather: expand segment-end values to a dense per-voxel-window
          layout (4096 voxels / window, 16 lanes share one index stream;
          untouched voxels read a guaranteed-zero pad slot)
        - DMA each dense window out contiguously
  host: place each window's exact voxel span into the zero output buffer.
"""

import numpy as np

from concourse import bacc, mybir, tile
from concourse.bass_utils import run_bass_kernel_spmd

B = 16
N_CELLS = 1048576
GRID_X, GRID_Y, GRID_Z = 320, 320, 80
N_VOX = GRID_X * GRID_Y * GRID_Z
N_CORES = 8
N_SLOTS = 8              # partition groups per core (16 lanes each)
STREAM = 16384           # cells per slot-stream (per partition free dim)
CELL_CAP = 16368         # leave >=16 pad cells per stream
PIECE_CELL_CAP = 12288
W = 4096                 # dense voxels per gather window
PIECE_SPAN_CAP = 16 * W  # max voxel span of one piece
ZSLOT = STREAM - 1       # guaranteed-zero stream position


def _build_plan(flat_idx):
    v = np.asarray(flat_idx, dtype=np.int64)
    order = np.argsort(v, kind="stable")
    sv = v[order]

    # segment boundaries in the sorted stream
    change = np.empty(N_CELLS, dtype=bool)
    change[0] = True
    change[1:] = sv[1:] != sv[:-1]
    seg_starts = np.flatnonzero(change)          # first cell of each segment
    n_seg = seg_starts.size
    seg_vox = sv[seg_starts]
    seg_ends = np.empty(n_seg, dtype=np.int64)   # last cell of each segment
    seg_ends[:-1] = seg_starts[1:] - 1
    seg_ends[-1] = N_CELLS - 1

    # cut into pieces at segment boundaries: cells <= PIECE_CELL_CAP, span <= PIECE_SPAN_CAP
    pieces = []  # (cell_a, cell_b, seg_a, seg_b)  [a,b) ranges
    sa = 0
    while sa < n_seg:
        lo_vox = seg_vox[sa]
        # furthest segment satisfying both caps
        sb = np.searchsorted(seg_vox, lo_vox + PIECE_SPAN_CAP, side="left")
        sb = min(sb, n_seg)
        ca = seg_starts[sa]
        # shrink by cell cap
        while seg_ends[sb - 1] + 1 - ca > PIECE_CELL_CAP:
            sb = sa + np.searchsorted(
                seg_ends[sa:sb] + 1 - ca, PIECE_CELL_CAP, side="right"
            )
        if sb <= sa:
            sb = sa + 1  # single huge segment (cells of one voxel) — must fit
        assert seg_ends[sb - 1] + 1 - ca <= CELL_CAP, (
            "a single voxel has more duplicate cells than one slot stream holds"
        )
        pieces.append((ca, seg_ends[sb - 1] + 1, sa, sb))
        sa = sb

    # windows per piece
    def piece_windows(p):
        ca, cb, sa_, sb_ = p
        span = seg_vox[sb_ - 1] - seg_vox[sa_] + 1
        return int(-(-span // W))

    # LPT pack pieces into bins of (round, core, slot)
    order_p = sorted(range(len(pieces)), key=lambda i: -piece_windows(pieces[i]))
    rounds = []  # rounds[r] = list of 64 bins; bin = {"cells":int,"wins":int,"pieces":[]}
    assign = {}

    def new_round():
        rounds.append(
            [{"cells": 0, "wins": 0, "pieces": []} for _ in range(N_CORES * N_SLOTS)]
        )

    new_round()
    for ip in order_p:
        p = pieces[ip]
        ncell = p[1] - p[0]
        wins = piece_windows(p)
        placed = False
        for r, bins in enumerate(rounds):
            cand = [b for b in bins if b["cells"] + ncell <= CELL_CAP]
            if cand:
                b = min(cand, key=lambda x: x["wins"])
                b["pieces"].append(ip)
                b["cells"] += ncell
                b["wins"] += wins
                assign[ip] = r
                placed = True
                break
        if not placed:
            new_round()
            b = rounds[-1][0]
            b["pieces"].append(ip)
            b["cells"] += ncell
            b["wins"] += wins
            assign[ip] = len(rounds) - 1

    R = len(rounds)
    WR = [max(max(b["wins"] for b in bins), 1) for bins in rounds]

    # per (round, core): cell source table, K mask, gather idx table, window spans
    plan = {"R": R, "WR": WR, "order": order, "per_core": []}
    for c in range(N_CORES):
        core_rounds = []
        for r in range(R):
            bins = rounds[r]
            wr = WR[r]
            cell_src = np.full((N_SLOTS, STREAM), -1, dtype=np.int64)
            kmask = np.ones((N_SLOTS, STREAM), dtype=np.float32)
            gidx = np.full((N_SLOTS, wr * W), ZSLOT, dtype=np.int32)
            spans = [[] for _ in range(N_SLOTS)]  # (window_index, vox_lo, length)
            for s in range(N_SLOTS):
                b = bins[c * N_SLOTS + s]
                pos = 0
                win = 0
                for ip in b["pieces"]:
                    ca, cb, sa_, sb_ = pieces[ip]
                    ncell = cb - ca
                    cell_src[s, pos : pos + ncell] = order[ca:cb]
                    # reset mask: 0 at each segment start
                    starts_local = seg_starts[sa_:sb_] - ca + pos
                    kmask[s, starts_local] = 0.0
                    # gather table: for each touched voxel, its segment end position
                    ends_local = seg_ends[sa_:sb_] - ca + pos
                    lo = seg_vox[sa_]
                    hi = seg_vox[sb_ - 1] + 1
                    span = hi - lo
                    nw = int(-(-span // W))
                    gbase = win * W
                    gidx[s, gbase + (seg_vox[sa_:sb_] - lo)] = ends_local
                    for wloc in range(nw):
                        vlo = lo + wloc * W
                        spans[s].append((win + wloc, vlo, int(min(W, hi - vlo))))
                    win += nw
                    pos += ncell
                # force zero at ZSLOT: reset + zero value (pad values are 0)
                kmask[s, ZSLOT] = 0.0
            core_rounds.append(
                {"cell_src": cell_src, "kmask": kmask, "gidx": gidx, "spans": spans}
            )
        plan["per_core"].append(core_rounds)
    return plan


def _wrap_idx(gidx_slot):
    """[NI] int -> wrapped [16, NI//16] int16 (j -> partition j%16, col j//16)."""
    ni = gidx_slot.size
    return np.ascontiguousarray(
        gidx_slot.astype(np.int16).reshape(ni // 16, 16).T
    )


def _build_nc(R, WR):
    nc = bacc.Bacc("TRN2", target_bir_lowering=False)
    ins, outs = [], []
    for r in range(R):
        wr = WR[r]
        ins.append(
            (
                nc.dram_tensor(f"vals{r}", (128, STREAM), mybir.dt.float32, kind="ExternalInput"),
                nc.dram_tensor(f"kmask{r}", (128, STREAM), mybir.dt.float32, kind="ExternalInput"),
                nc.dram_tensor(f"gidx{r}", (128, (wr * W) // 16), mybir.dt.int16, kind="ExternalInput"),
            )
        )
        outs.append(
            nc.dram_tensor(f"dense{r}", (128, wr * W), mybir.dt.float32, kind="ExternalOutput")
        )
    with tile.TileContext(nc) as tc:
        with tc.tile_pool(name="sbuf", bufs=1) as pool:
            for r in range(R):
                vals_d, kmask_d, gidx_d = ins[r]
                wr = WR[r]
                v = pool.tile([128, STREAM], mybir.dt.float32, tag="vals")
                k = pool.tile([128, STREAM], mybir.dt.float32, tag="kmask")
                g = pool.tile([128, (wr * W) // 16], mybir.dt.int16, tag="gidx")
                scan = pool.tile([128, STREAM], mybir.dt.float32, tag="scan")
                nc.sync.dma_start(v[:], vals_d[:])
                nc.sync.dma_start(k[:], kmask_d[:])
                nc.sync.dma_start(g[:], gidx_d[:])
                nc.vector.tensor_tensor_scan(
                    scan[:], k[:], v[:], 0.0,
                    op0=mybir.AluOpType.mult, op1=mybir.AluOpType.add,
                )
                for w in range(wr):
                    # vals/kmask slots are dead after the scan; reuse them as
                    # alternating gather-output buffers (double buffering)
                    go = pool.tile([128, W], mybir.dt.float32,
                                   tag="vals" if w % 2 == 0 else "kmask")
                    nc.gpsimd.ap_gather(
                        go[:], scan[:], g[:, (w * W) // 16 : ((w + 1) * W) // 16],
                        channels=128, num_elems=STREAM, d=1, num_idxs=W,
                    )
                    nc.sync.dma_start(outs[r][:, w * W : (w + 1) * W], go[:])
    nc.compile()
    return nc


_CACHE = {}


def kernel(polar_frames, flat_voxel_indices):
    polar = np.asarray(polar_frames, dtype=np.float32).reshape(B, N_CELLS)
    idx_key = np.asarray(flat_voxel_indices).tobytes()[:256]  # cheap cache key
    if idx_key in _CACHE:
        plan, nc = _CACHE[idx_key]
    else:
        plan = _build_plan(flat_voxel_indices)
        nc = _build_nc(plan["R"], plan["WR"])
        _CACHE[idx_key] = (plan, nc)

    R, WR = plan["R"], plan["WR"]
    in_maps = []
    for c in range(N_CORES):
        m = {}
        for r in range(R):
            pc = plan["per_core"][c][r]
            cell_src = pc["cell_src"]  # [8, STREAM] int64, -1 = pad
            vals = np.zeros((N_SLOTS, B, STREAM), dtype=np.float32)
            valid = cell_src >= 0
            for s in range(N_SLOTS):
                vs = valid[s]
                vals[s, :, vs] = polar[:, cell_src[s, vs]].T
            m[f"vals{r}"] = vals.reshape(128, STREAM)
            m[f"kmask{r}"] = np.repeat(pc["kmask"], B, axis=0).reshape(128, STREAM)
            gw = np.zeros((N_SLOTS, 16, (WR[r] * W) // 16), dtype=np.int16)
            for s in range(N_SLOTS):
                gw[s] = _wrap_idx(pc["gidx"][s])
            m[f"gidx{r}"] = gw.reshape(128, (WR[r] * W) // 16)
        in_maps.append(m)

    res = run_bass_kernel_spmd(nc, in_maps, core_ids=list(range(N_CORES)))

    out = np.zeros((B, N_VOX), dtype=np.float32)
    for c in range(N_CORES):
        for r in range(R):
            dense = res.results[c][f"dense{r}"].reshape(N_SLOTS, B, WR[r] * W)
            pc = plan["per_core"][c][r]
            for s in range(N_SLOTS):
                for (win, vlo, ln) in pc["spans"][s]:
                    out[:, vlo : vlo + ln] = dense[s, :, win * W : win * W + ln]
    return out.reshape(B, 1, GRID_Z, GRID_Y, GRID_X)


# revision 6
# speedup vs baseline: 2.7907x; 2.7907x over previous
"""PolarToCartesianGrid scatter-add kernel for 8 Trainium2 NeuronCores.

Strategy (voxel-range sharded, all 16 batch samples as partition lanes):
  host: sort polar cells by target voxel (indices are compile-time data);
        cut the sorted stream into 2048-cell window-slices (each covering a
        <=4096-voxel span of segment ends); oversized segments become chains
        of full "feeder" slices whose running sum continues into the next
        slice; LPT-pack chains into 64 slot-streams (8 cores x 8 groups).
  device (per core, per round):
        - DMA in values [128, WR*2048] fp32 (partition = 16*slot + lane)
        - tensor_tensor_scan(mult,add) with a bf16 reset mask => running
          segment sums; each voxel's total sits at its segment-end position
        - per window w: ap_gather from ONLY the 2048-cell slice w (small
          source window => ~3.5us/window instead of 23us) expanding segment
          ends to a dense 4096-voxel layout; untouched voxels read a
          guaranteed-zero pad slot of the slice
        - DMA each dense window out contiguously
  host: place each window's exact voxel span into the zero output buffer.
"""

import numpy as np
import ml_dtypes

from concourse import bacc, mybir, tile
from concourse.bass_utils import run_bass_kernel_spmd

B = 16
N_CELLS = 1048576
GRID_X, GRID_Y, GRID_Z = 320, 320, 80
N_VOX = GRID_X * GRID_Y * GRID_Z
N_CORES = 8
N_SLOTS = 8          # partition groups per core (16 lanes each)
SLICE = 2048         # stream cells per window-slice
SLICE_CAP = 2047     # normal slices reserve >=1 zero pad
W = 4096             # dense voxels out per window
WPR = 8              # max windows per slot per round (SBUF bound)


class _Slice:
    __slots__ = ("cells", "kvec", "ends", "zslot", "used")

    def __init__(self):
        self.cells = np.full(SLICE, -1, dtype=np.int64)
        self.kvec = np.zeros(SLICE, dtype=np.float32)  # pads: K=0 (reset), v=0
        self.ends = []  # (local_pos, voxel)
        self.zslot = 0
        self.used = 0


def _build_plan(flat_idx):
    v = np.asarray(flat_idx, dtype=np.int64)
    order = np.argsort(v, kind="stable")
    sv = v[order]

    change = np.empty(N_CELLS, dtype=bool)
    change[0] = True
    change[1:] = sv[1:] != sv[:-1]
    seg_starts = np.flatnonzero(change)
    n_seg = seg_starts.size
    seg_vox = sv[seg_starts]
    seg_cnt = np.empty(n_seg, dtype=np.int64)
    seg_cnt[:-1] = np.diff(seg_starts)
    seg_cnt[-1] = N_CELLS - seg_starts[-1]

    chains = []          # list of lists of _Slice
    cur_chain = [None]   # box for current chain under construction

    def new_slice(linked):
        s = _Slice()
        if linked and cur_chain[0] is not None:
            cur_chain[0].append(s)
        else:
            cur_chain[0] = [s]
            chains.append(cur_chain[0])
        return s

    def finalize(s):
        s.zslot = s.used if s.used < SLICE else 0

    cur = new_slice(False)
    for i in range(n_seg):
        cnt = int(seg_cnt[i])
        vox = int(seg_vox[i])
        ca = int(seg_starts[i])
        if cnt <= SLICE_CAP:
            fits_cells = cur.used + cnt <= SLICE_CAP
            fits_span = (not cur.ends) or (vox - cur.ends[0][1] < W)
            if not (fits_cells and fits_span):
                finalize(cur)
                cur = new_slice(False)
            p = cur.used
            cur.cells[p : p + cnt] = order[ca : ca + cnt]
            cur.kvec[p] = 0.0
            cur.kvec[p + 1 : p + cnt] = 1.0
            cur.ends.append((p + cnt - 1, vox))
            cur.used += cnt
        else:
            # oversized segment: close current, emit full feeder slices whose
            # running sum continues into the chain's tail slice
            if cur.used > 0 or cur.ends:
                finalize(cur)
                cur = new_slice(False)
            remaining = cnt
            off = ca
            first = True
            while remaining > SLICE_CAP:
                take = min(SLICE, remaining - 1)  # keep >=1 cell for the tail
                f = cur if first else new_slice(True)
                f.cells[:take] = order[off : off + take]
                f.kvec[:] = 1.0  # trailing pads: K=1, v=0 (continue, add 0)
                if first:
                    f.kvec[0] = 0.0
                f.used = SLICE
                f.zslot = 0  # feeder: no outputs gathered
                cur = f
                remaining -= take
                off += take
                first = False
            tail = new_slice(True)
            tail.cells[:remaining] = order[off : off + remaining]
            tail.kvec[0] = 1.0  # continues the feeder chain's running sum
            tail.kvec[1:remaining] = 1.0
            tail.ends.append((remaining - 1, vox))
            tail.used = remaining
            cur = tail
    finalize(cur)

    for ch in chains:
        assert len(ch) <= WPR, "segment chain exceeds one round's window budget"

    # LPT: assign chains to 64 slots balancing window counts, then split each
    # slot's chain list into rounds of <= WPR windows
    slot_loads = [0] * (N_CORES * N_SLOTS)
    slot_chains = [[] for _ in range(N_CORES * N_SLOTS)]
    for ch in sorted(chains, key=len, reverse=True):
        s = int(np.argmin(slot_loads))
        slot_chains[s].append(ch)
        slot_loads[s] += len(ch)

    slot_rounds = []  # [slot] -> list of rounds, each a list of _Slice
    for s in range(N_CORES * N_SLOTS):
        rounds = [[]]
        for ch in slot_chains[s]:
            if len(rounds[-1]) + len(ch) > WPR:
                rounds.append([])
            rounds[-1].extend(ch)
        slot_rounds.append(rounds)

    R = max(len(r) for r in slot_rounds)
    WR = []
    for r in range(R):
        WR.append(
            max(
                (len(sr[r]) if r < len(sr) else 0)
                for sr in slot_rounds
            )
        )
    WR = [max(w, 1) for w in WR]

    plan = {"R": R, "WR": WR, "per_core": []}
    empty = _Slice()
    finalize(empty)
    for c in range(N_CORES):
        core_rounds = []
        for r in range(R):
            wr = WR[r]
            cell_src = np.full((N_SLOTS, wr * SLICE), -1, dtype=np.int64)
            kmask = np.zeros((N_SLOTS, wr * SLICE), dtype=np.float32)
            gidx = np.zeros((N_SLOTS, wr * W), dtype=np.int32)
            spans = [[] for _ in range(N_SLOTS)]  # (win, vox_lo, len)
            for s in range(N_SLOTS):
                sr = slot_rounds[c * N_SLOTS + s]
                wins = sr[r] if r < len(sr) else []
                for w in range(wr):
                    sl = wins[w] if w < len(wins) else empty
                    cell_src[s, w * SLICE : (w + 1) * SLICE] = sl.cells
                    kmask[s, w * SLICE : (w + 1) * SLICE] = sl.kvec
                    gidx[s, w * W : (w + 1) * W] = sl.zslot
                    if sl.ends:
                        lo = sl.ends[0][1]
                        hi = sl.ends[-1][1] + 1
                        for (pos, vox) in sl.ends:
                            gidx[s, w * W + (vox - lo)] = pos
                        spans[s].append((w, lo, hi - lo))
            core_rounds.append(
                {"cell_src": cell_src, "kmask": kmask, "gidx": gidx, "spans": spans}
            )
        plan["per_core"].append(core_rounds)
    return plan


def _wrap_idx(gidx_slot):
    """[NI] int -> wrapped [16, NI//16] int16 (idx j -> partition j%16, col j//16)."""
    ni = gidx_slot.size
    return np.ascontiguousarray(gidx_slot.astype(np.int16).reshape(ni // 16, 16).T)


def _build_nc(R, WR):
    nc = bacc.Bacc("TRN2", target_bir_lowering=False)
    ins, outs = [], []
    for r in range(R):
        ns = WR[r] * SLICE
        ins.append(
            (
                nc.dram_tensor(f"vals{r}", (128, ns), mybir.dt.float32, kind="ExternalInput"),
                nc.dram_tensor(f"kmask{r}", (128, ns), mybir.dt.bfloat16, kind="ExternalInput"),
                nc.dram_tensor(f"gidx{r}", (128, (WR[r] * W) // 16), mybir.dt.int16, kind="ExternalInput"),
            )
        )
        outs.append(
            nc.dram_tensor(f"dense{r}", (128, WR[r] * W), mybir.dt.float32, kind="ExternalOutput")
        )
    with tile.TileContext(nc) as tc:
        with tc.tile_pool(name="sbuf", bufs=1) as pool, tc.tile_pool(name="io", bufs=2) as iop:
            for r in range(R):
                vals_d, kmask_d, gidx_d = ins[r]
                wr = WR[r]
                ns = wr * SLICE
                v = pool.tile([128, ns], mybir.dt.float32, tag="vals")
                k = pool.tile([128, ns], mybir.dt.bfloat16, tag="kmask")
                g = pool.tile([128, (wr * W) // 16], mybir.dt.int16, tag="gidx")
                scan = pool.tile([128, ns], mybir.dt.float32, tag="scan")
                nc.sync.dma_start(v[:], vals_d[:])
                nc.sync.dma_start(k[:], kmask_d[:])
                nc.sync.dma_start(g[:], gidx_d[:])
                nc.vector.tensor_tensor_scan(
                    scan[:], k[:], v[:], 0.0,
                    op0=mybir.AluOpType.mult, op1=mybir.AluOpType.add,
                )
                for w in range(wr):
                    go = iop.tile([128, W], mybir.dt.float32, tag="gout")
                    nc.gpsimd.ap_gather(
                        go[:],
                        scan[:, w * SLICE : (w + 1) * SLICE],
                        g[:, (w * W) // 16 : ((w + 1) * W) // 16],
                        channels=128, num_elems=SLICE, d=1, num_idxs=W,
                    )
                    nc.sync.dma_start(outs[r][:, w * W : (w + 1) * W], go[:])
    nc.compile()
    return nc


_CACHE = {}


def kernel(polar_frames, flat_voxel_indices):
    polar = np.asarray(polar_frames, dtype=np.float32).reshape(B, N_CELLS)
    idx_key = np.asarray(flat_voxel_indices).tobytes()[:256]
    if idx_key in _CACHE:
        plan, nc = _CACHE[idx_key]
    else:
        plan = _build_plan(flat_voxel_indices)
        nc = _build_nc(plan["R"], plan["WR"])
        _CACHE[idx_key] = (plan, nc)

    R, WR = plan["R"], plan["WR"]
    in_maps = []
    for c in range(N_CORES):
        m = {}
        for r in range(R):
            pc = plan["per_core"][c][r]
            ns = WR[r] * SLICE
            cell_src = pc["cell_src"]  # [8, ns], -1 = pad
            vals = np.zeros((N_SLOTS, B, ns), dtype=np.float32)
            valid = cell_src >= 0
            for s in range(N_SLOTS):
                vs = valid[s]
                vals[s, :, vs] = polar[:, cell_src[s, vs]].T
            m[f"vals{r}"] = vals.reshape(128, ns)
            m[f"kmask{r}"] = np.repeat(pc["kmask"], B, axis=0).reshape(128, ns).astype(
                ml_dtypes.bfloat16
            )
            gw = np.zeros((N_SLOTS, 16, (WR[r] * W) // 16), dtype=np.int16)
            for s in range(N_SLOTS):
                gw[s] = _wrap_idx(pc["gidx"][s])
            m[f"gidx{r}"] = gw.reshape(128, (WR[r] * W) // 16)
        in_maps.append(m)

    res = run_bass_kernel_spmd(nc, in_maps, core_ids=list(range(N_CORES)))

    out = np.zeros((B, N_VOX), dtype=np.float32)
    for c in range(N_CORES):
        for r in range(R):
            dense = res.results[c][f"dense{r}"].reshape(N_SLOTS, B, WR[r] * W)
            pc = plan["per_core"][c][r]
            for s in range(N_SLOTS):
                for (win, vlo, ln) in pc["spans"][s]:
                    out[:, vlo : vlo + ln] = dense[s, :, win * W : win * W + ln]
    return out.reshape(B, 1, GRID_Z, GRID_Y, GRID_X)


# revision 7
# speedup vs baseline: 3.1644x; 1.1339x over previous
"""PolarToCartesianGrid scatter-add kernel for 8 Trainium2 NeuronCores.

Strategy (voxel-range sharded, all 16 batch samples as partition lanes):
  host: sort polar cells by target voxel (indices are compile-time data);
        cut the sorted stream into 2048-cell window-slices (each covering a
        <=4096-voxel span of segment ends); oversized segments become chains
        of full "feeder" slices whose running sum continues into the next
        slice; LPT-pack chains into 64 slot-streams (8 cores x 8 groups).
  device (per core, per round):
        - DMA in values [128, WR*2048] fp32 (partition = 16*slot + lane)
        - tensor_tensor_scan(mult,add) with a bf16 reset mask => running
          segment sums; each voxel's total sits at its segment-end position
        - per window w: ap_gather from ONLY the 2048-cell slice w (small
          source window => ~3.5us/window instead of 23us) expanding segment
          ends to a dense 4096-voxel layout; untouched voxels read a
          guaranteed-zero pad slot of the slice
        - DMA each dense window out contiguously
  host: place each window's exact voxel span into the zero output buffer.
"""

import numpy as np
import ml_dtypes

from concourse import bacc, mybir, tile
from concourse.bass_utils import run_bass_kernel_spmd

B = 16
N_CELLS = 1048576
GRID_X, GRID_Y, GRID_Z = 320, 320, 80
N_VOX = GRID_X * GRID_Y * GRID_Z
N_CORES = 8
N_SLOTS = 8          # partition groups per core (16 lanes each)
SLICE = 2048         # stream cells per window-slice
SLICE_CAP = 2047     # normal slices reserve >=1 zero pad
W = 4096             # dense voxels out per window
WPR = 4              # max windows per slot per round (SBUF bound)


class _Slice:
    __slots__ = ("cells", "kvec", "ends", "zslot", "used")

    def __init__(self):
        self.cells = np.full(SLICE, -1, dtype=np.int64)
        self.kvec = np.zeros(SLICE, dtype=np.float32)  # pads: K=0 (reset), v=0
        self.ends = []  # (local_pos, voxel)
        self.zslot = 0
        self.used = 0


def _build_plan(flat_idx):
    v = np.asarray(flat_idx, dtype=np.int64)
    order = np.argsort(v, kind="stable")
    sv = v[order]

    change = np.empty(N_CELLS, dtype=bool)
    change[0] = True
    change[1:] = sv[1:] != sv[:-1]
    seg_starts = np.flatnonzero(change)
    n_seg = seg_starts.size
    seg_vox = sv[seg_starts]
    seg_cnt = np.empty(n_seg, dtype=np.int64)
    seg_cnt[:-1] = np.diff(seg_starts)
    seg_cnt[-1] = N_CELLS - seg_starts[-1]

    chains = []          # list of lists of _Slice
    cur_chain = [None]   # box for current chain under construction

    def new_slice(linked):
        s = _Slice()
        if linked and cur_chain[0] is not None:
            cur_chain[0].append(s)
        else:
            cur_chain[0] = [s]
            chains.append(cur_chain[0])
        return s

    def finalize(s):
        s.zslot = s.used if s.used < SLICE else 0

    cur = new_slice(False)
    for i in range(n_seg):
        cnt = int(seg_cnt[i])
        vox = int(seg_vox[i])
        ca = int(seg_starts[i])
        if cnt <= SLICE_CAP:
            fits_cells = cur.used + cnt <= SLICE_CAP
            fits_span = (not cur.ends) or (vox - cur.ends[0][1] < W)
            if not (fits_cells and fits_span):
                finalize(cur)
                cur = new_slice(False)
            p = cur.used
            cur.cells[p : p + cnt] = order[ca : ca + cnt]
            cur.kvec[p] = 0.0
            cur.kvec[p + 1 : p + cnt] = 1.0
            cur.ends.append((p + cnt - 1, vox))
            cur.used += cnt
        else:
            # oversized segment: close current, emit full feeder slices whose
            # running sum continues into the chain's tail slice
            if cur.used > 0 or cur.ends:
                finalize(cur)
                cur = new_slice(False)
            remaining = cnt
            off = ca
            first = True
            while remaining > SLICE_CAP:
                take = min(SLICE, remaining - 1)  # keep >=1 cell for the tail
                f = cur if first else new_slice(True)
                f.cells[:take] = order[off : off + take]
                f.kvec[:] = 1.0  # trailing pads: K=1, v=0 (continue, add 0)
                if first:
                    f.kvec[0] = 0.0
                f.used = SLICE
                f.zslot = 0  # feeder: no outputs gathered
                cur = f
                remaining -= take
                off += take
                first = False
            tail = new_slice(True)
            tail.cells[:remaining] = order[off : off + remaining]
            tail.kvec[0] = 1.0  # continues the feeder chain's running sum
            tail.kvec[1:remaining] = 1.0
            tail.ends.append((remaining - 1, vox))
            tail.used = remaining
            cur = tail
    finalize(cur)

    for ch in chains:
        assert len(ch) <= WPR, "segment chain exceeds one round's window budget"

    # LPT: assign chains to 64 slots balancing window counts, then split each
    # slot's chain list into rounds of <= WPR windows
    slot_loads = [0] * (N_CORES * N_SLOTS)
    slot_chains = [[] for _ in range(N_CORES * N_SLOTS)]
    for ch in sorted(chains, key=len, reverse=True):
        s = int(np.argmin(slot_loads))
        slot_chains[s].append(ch)
        slot_loads[s] += len(ch)

    slot_rounds = []  # [slot] -> list of rounds, each a list of _Slice
    for s in range(N_CORES * N_SLOTS):
        rounds = [[]]
        for ch in slot_chains[s]:
            if len(rounds[-1]) + len(ch) > WPR:
                rounds.append([])
            rounds[-1].extend(ch)
        slot_rounds.append(rounds)

    R = max(len(r) for r in slot_rounds)
    WR = []
    for r in range(R):
        WR.append(
            max(
                (len(sr[r]) if r < len(sr) else 0)
                for sr in slot_rounds
            )
        )
    WR = [max(w, 1) for w in WR]

    plan = {"R": R, "WR": WR, "per_core": []}
    empty = _Slice()
    finalize(empty)
    for c in range(N_CORES):
        core_rounds = []
        for r in range(R):
            wr = WR[r]
            cell_src = np.full((N_SLOTS, wr * SLICE), -1, dtype=np.int64)
            kmask = np.zeros((N_SLOTS, wr * SLICE), dtype=np.float32)
            gidx = np.zeros((N_SLOTS, wr * W), dtype=np.int32)
            spans = [[] for _ in range(N_SLOTS)]  # (win, vox_lo, len)
            for s in range(N_SLOTS):
                sr = slot_rounds[c * N_SLOTS + s]
                wins = sr[r] if r < len(sr) else []
                for w in range(wr):
                    sl = wins[w] if w < len(wins) else empty
                    cell_src[s, w * SLICE : (w + 1) * SLICE] = sl.cells
                    kmask[s, w * SLICE : (w + 1) * SLICE] = sl.kvec
                    gidx[s, w * W : (w + 1) * W] = sl.zslot
                    if sl.ends:
                        lo = sl.ends[0][1]
                        hi = sl.ends[-1][1] + 1
                        for (pos, vox) in sl.ends:
                            gidx[s, w * W + (vox - lo)] = pos
                        spans[s].append((w, lo, hi - lo))
            core_rounds.append(
                {"cell_src": cell_src, "kmask": kmask, "gidx": gidx, "spans": spans}
            )
        plan["per_core"].append(core_rounds)
    return plan


def _wrap_idx(gidx_slot):
    """[NI] int -> wrapped [16, NI//16] int16 (idx j -> partition j%16, col j//16)."""
    ni = gidx_slot.size
    return np.ascontiguousarray(gidx_slot.astype(np.int16).reshape(ni // 16, 16).T)


def _build_nc(R, WR):
    nc = bacc.Bacc("TRN2", target_bir_lowering=False)
    ins, outs = [], []
    for r in range(R):
        ns = WR[r] * SLICE
        ins.append(
            (
                nc.dram_tensor(f"vals{r}", (128, ns), mybir.dt.float32, kind="ExternalInput"),
                nc.dram_tensor(f"kmask{r}", (128, ns), mybir.dt.bfloat16, kind="ExternalInput"),
                nc.dram_tensor(f"gidx{r}", (128, (WR[r] * W) // 16), mybir.dt.int16, kind="ExternalInput"),
            )
        )
        outs.append(
            nc.dram_tensor(f"dense{r}", (128, WR[r] * W), mybir.dt.float32, kind="ExternalOutput")
        )
    with tile.TileContext(nc) as tc:
        with tc.tile_pool(name="sbuf", bufs=2) as pool, tc.tile_pool(name="io", bufs=2) as iop:
            for r in range(R):
                vals_d, kmask_d, gidx_d = ins[r]
                wr = WR[r]
                ns = wr * SLICE
                v = pool.tile([128, ns], mybir.dt.float32, tag="vals")
                k = pool.tile([128, ns], mybir.dt.bfloat16, tag="kmask")
                g = pool.tile([128, (wr * W) // 16], mybir.dt.int16, tag="gidx")
                scan = pool.tile([128, ns], mybir.dt.float32, tag="scan")
                nc.sync.dma_start(v[:], vals_d[:])
                nc.sync.dma_start(k[:], kmask_d[:])
                nc.sync.dma_start(g[:], gidx_d[:])
                nc.vector.tensor_tensor_scan(
                    scan[:], k[:], v[:], 0.0,
                    op0=mybir.AluOpType.mult, op1=mybir.AluOpType.add,
                )
                for w in range(wr):
                    go = iop.tile([128, W], mybir.dt.float32, tag="gout")
                    nc.gpsimd.ap_gather(
                        go[:],
                        scan[:, w * SLICE : (w + 1) * SLICE],
                        g[:, (w * W) // 16 : ((w + 1) * W) // 16],
                        channels=128, num_elems=SLICE, d=1, num_idxs=W,
                    )
                    nc.sync.dma_start(outs[r][:, w * W : (w + 1) * W], go[:])
    nc.compile()
    return nc


_CACHE = {}


def kernel(polar_frames, flat_voxel_indices):
    polar = np.asarray(polar_frames, dtype=np.float32).reshape(B, N_CELLS)
    idx_key = np.asarray(flat_voxel_indices).tobytes()[:256]
    if idx_key in _CACHE:
        plan, nc = _CACHE[idx_key]
    else:
        plan = _build_plan(flat_voxel_indices)
        nc = _build_nc(plan["R"], plan["WR"])
        _CACHE[idx_key] = (plan, nc)

    R, WR = plan["R"], plan["WR"]
    in_maps = []
    for c in range(N_CORES):
        m = {}
        for r in range(R):
            pc = plan["per_core"][c][r]
            ns = WR[r] * SLICE
            cell_src = pc["cell_src"]  # [8, ns], -1 = pad
            vals = np.zeros((N_SLOTS, B, ns), dtype=np.float32)
            valid = cell_src >= 0
            for s in range(N_SLOTS):
                vs = valid[s]
                vals[s, :, vs] = polar[:, cell_src[s, vs]].T
            m[f"vals{r}"] = vals.reshape(128, ns)
            m[f"kmask{r}"] = np.repeat(pc["kmask"], B, axis=0).reshape(128, ns).astype(
                ml_dtypes.bfloat16
            )
            gw = np.zeros((N_SLOTS, 16, (WR[r] * W) // 16), dtype=np.int16)
            for s in range(N_SLOTS):
                gw[s] = _wrap_idx(pc["gidx"][s])
            m[f"gidx{r}"] = gw.reshape(128, (WR[r] * W) // 16)
        in_maps.append(m)

    res = run_bass_kernel_spmd(nc, in_maps, core_ids=list(range(N_CORES)))

    out = np.zeros((B, N_VOX), dtype=np.float32)
    for c in range(N_CORES):
        for r in range(R):
            dense = res.results[c][f"dense{r}"].reshape(N_SLOTS, B, WR[r] * W)
            pc = plan["per_core"][c][r]
            for s in range(N_SLOTS):
                for (win, vlo, ln) in pc["spans"][s]:
                    out[:, vlo : vlo + ln] = dense[s, :, win * W : win * W + ln]
    return out.reshape(B, 1, GRID_Z, GRID_Y, GRID_X)


# revision 8
# speedup vs baseline: 3.3283x; 1.0518x over previous
"""PolarToCartesianGrid scatter-add kernel for 8 Trainium2 NeuronCores.

Strategy (voxel-range sharded, all 16 batch samples as partition lanes):
  host: sort polar cells by target voxel (indices are compile-time data);
        cut the sorted stream into 2048-cell window-slices (each covering a
        <=4096-voxel span of segment ends); oversized segments become chains
        of full "feeder" slices whose running sum continues into the next
        slice; LPT-pack chains into 64 slot-streams (8 cores x 8 groups).
  device (per core, per round):
        - DMA in values [128, WR*2048] fp32 (partition = 16*slot + lane)
        - tensor_tensor_scan(mult,add) with a bf16 reset mask => running
          segment sums; each voxel's total sits at its segment-end position
        - per window w: ap_gather from ONLY the 2048-cell slice w (small
          source window => ~3.5us/window instead of 23us) expanding segment
          ends to a dense 4096-voxel layout; untouched voxels read a
          guaranteed-zero pad slot of the slice
        - DMA each dense window out contiguously
  host: place each window's exact voxel span into the zero output buffer.
"""

import numpy as np

from concourse import bacc, mybir, tile
from concourse.bass_utils import run_bass_kernel_spmd

B = 16
N_CELLS = 1048576
GRID_X, GRID_Y, GRID_Z = 320, 320, 80
N_VOX = GRID_X * GRID_Y * GRID_Z
N_CORES = 8
N_SLOTS = 8          # partition groups per core (16 lanes each)
SLICE = 2048         # stream cells per window-slice
SLICE_CAP = 2047     # normal slices reserve >=1 zero pad
W = 4096             # dense voxels out per window
WPR = 4              # max windows per slot per round (SBUF bound)


class _Slice:
    __slots__ = ("cells", "kvec", "ends", "zslot", "used")

    def __init__(self):
        self.cells = np.full(SLICE, -1, dtype=np.int64)
        self.kvec = np.zeros(SLICE, dtype=np.float32)  # pads: K=0 (reset), v=0
        self.ends = []  # (local_pos, voxel)
        self.zslot = 0
        self.used = 0


def _build_plan(flat_idx):
    v = np.asarray(flat_idx, dtype=np.int64)
    order = np.argsort(v, kind="stable")
    sv = v[order]

    change = np.empty(N_CELLS, dtype=bool)
    change[0] = True
    change[1:] = sv[1:] != sv[:-1]
    seg_starts = np.flatnonzero(change)
    n_seg = seg_starts.size
    seg_vox = sv[seg_starts]
    seg_cnt = np.empty(n_seg, dtype=np.int64)
    seg_cnt[:-1] = np.diff(seg_starts)
    seg_cnt[-1] = N_CELLS - seg_starts[-1]

    chains = []          # list of lists of _Slice
    cur_chain = [None]   # box for current chain under construction

    def new_slice(linked):
        s = _Slice()
        if linked and cur_chain[0] is not None:
            cur_chain[0].append(s)
        else:
            cur_chain[0] = [s]
            chains.append(cur_chain[0])
        return s

    def finalize(s):
        s.zslot = s.used if s.used < SLICE else 0

    cur = new_slice(False)
    for i in range(n_seg):
        cnt = int(seg_cnt[i])
        vox = int(seg_vox[i])
        ca = int(seg_starts[i])
        if cnt <= SLICE_CAP:
            fits_cells = cur.used + cnt <= SLICE_CAP
            fits_span = (not cur.ends) or (vox - cur.ends[0][1] < W)
            if not (fits_cells and fits_span):
                finalize(cur)
                cur = new_slice(False)
            p = cur.used
            cur.cells[p : p + cnt] = order[ca : ca + cnt]
            cur.kvec[p] = 0.0
            cur.kvec[p + 1 : p + cnt] = 1.0
            cur.ends.append((p + cnt - 1, vox))
            cur.used += cnt
        else:
            # oversized segment: close current, emit full feeder slices whose
            # running sum continues into the chain's tail slice
            if cur.used > 0 or cur.ends:
                finalize(cur)
                cur = new_slice(False)
            remaining = cnt
            off = ca
            first = True
            while remaining > SLICE_CAP:
                take = min(SLICE, remaining - 1)  # keep >=1 cell for the tail
                f = cur if first else new_slice(True)
                f.cells[:take] = order[off : off + take]
                f.kvec[:] = 1.0  # trailing pads: K=1, v=0 (continue, add 0)
                if first:
                    f.kvec[0] = 0.0
                f.used = SLICE
                f.zslot = 0  # feeder: no outputs gathered
                cur = f
                remaining -= take
                off += take
                first = False
            tail = new_slice(True)
            tail.cells[:remaining] = order[off : off + remaining]
            tail.kvec[0] = 1.0  # continues the feeder chain's running sum
            tail.kvec[1:remaining] = 1.0
            tail.ends.append((remaining - 1, vox))
            tail.used = remaining
            cur = tail
    finalize(cur)

    for ch in chains:
        assert len(ch) <= WPR, "segment chain exceeds one round's window budget"

    # LPT: assign chains to 64 slots balancing window counts, then split each
    # slot's chain list into rounds of <= WPR windows
    slot_loads = [0] * (N_CORES * N_SLOTS)
    slot_chains = [[] for _ in range(N_CORES * N_SLOTS)]
    for ch in sorted(chains, key=len, reverse=True):
        s = int(np.argmin(slot_loads))
        slot_chains[s].append(ch)
        slot_loads[s] += len(ch)

    slot_rounds = []  # [slot] -> list of rounds, each a list of _Slice
    for s in range(N_CORES * N_SLOTS):
        rounds = [[]]
        for ch in slot_chains[s]:
            if len(rounds[-1]) + len(ch) > WPR:
                rounds.append([])
            rounds[-1].extend(ch)
        slot_rounds.append(rounds)

    R = max(len(r) for r in slot_rounds)
    WR = []
    for r in range(R):
        WR.append(
            max(
                (len(sr[r]) if r < len(sr) else 0)
                for sr in slot_rounds
            )
        )
    WR = [max(w, 1) for w in WR]

    plan = {"R": R, "WR": WR, "per_core": [], "trims": []}
    empty = _Slice()
    finalize(empty)
    for c in range(N_CORES):
        core_rounds = []
        for r in range(R):
            wr = WR[r]
            cell_src = np.full((N_SLOTS, wr * SLICE), -1, dtype=np.int64)
            kmask = np.zeros((N_SLOTS, wr * SLICE), dtype=np.float32)
            gidx = np.zeros((N_SLOTS, wr * W), dtype=np.int32)
            spans = [[] for _ in range(N_SLOTS)]  # (win, vox_lo, len)
            for s in range(N_SLOTS):
                sr = slot_rounds[c * N_SLOTS + s]
                wins = sr[r] if r < len(sr) else []
                for w in range(wr):
                    sl = wins[w] if w < len(wins) else empty
                    cell_src[s, w * SLICE : (w + 1) * SLICE] = sl.cells
                    kmask[s, w * SLICE : (w + 1) * SLICE] = sl.kvec
                    gidx[s, w * W : (w + 1) * W] = sl.zslot
                    if sl.ends:
                        lo = sl.ends[0][1]
                        hi = sl.ends[-1][1] + 1
                        for (pos, vox) in sl.ends:
                            gidx[s, w * W + (vox - lo)] = pos
                        spans[s].append((w, lo, hi - lo))
            core_rounds.append(
                {"cell_src": cell_src, "kmask": kmask, "gidx": gidx, "spans": spans}
            )
        plan["per_core"].append(core_rounds)
    # per (round, window): max span across all cores/slots, rounded up to 16
    trims = []
    for r in range(R):
        tr = []
        for w in range(WR[r]):
            mx = 16
            for c in range(N_CORES):
                for sp in plan["per_core"][c][r]["spans"]:
                    for (win, _lo, ln) in sp:
                        if win == w:
                            mx = max(mx, ln)
            tr.append(min(W, -(-mx // 16) * 16))
        trims.append(tr)
    plan["trims"] = trims
    return plan


def _wrap_idx(gidx_slot):
    """[NI] int -> wrapped [16, NI//16] int16 (idx j -> partition j%16, col j//16)."""
    ni = gidx_slot.size
    return np.ascontiguousarray(gidx_slot.astype(np.int16).reshape(ni // 16, 16).T)


def _build_nc(R, WR, trims):
    nc = bacc.Bacc("TRN2", target_bir_lowering=False)
    ins, outs = [], []
    for r in range(R):
        ns = WR[r] * SLICE
        ins.append(
            (
                nc.dram_tensor(f"vals{r}", (128, ns), mybir.dt.float32, kind="ExternalInput"),
                nc.dram_tensor(f"kmask{r}", (128, ns), mybir.dt.uint8, kind="ExternalInput"),
                nc.dram_tensor(f"gidx{r}", (128, (WR[r] * W) // 16), mybir.dt.int16, kind="ExternalInput"),
            )
        )
        outs.append(
            nc.dram_tensor(f"dense{r}", (128, WR[r] * W), mybir.dt.float32, kind="ExternalOutput")
        )
    with tile.TileContext(nc) as tc:
        with tc.tile_pool(name="sbuf", bufs=2) as pool, tc.tile_pool(name="io", bufs=2) as iop:
            for r in range(R):
                vals_d, kmask_d, gidx_d = ins[r]
                wr = WR[r]
                ns = wr * SLICE
                v = pool.tile([128, ns], mybir.dt.float32, tag="vals")
                k = pool.tile([128, ns], mybir.dt.uint8, tag="kmask")
                g = pool.tile([128, (wr * W) // 16], mybir.dt.int16, tag="gidx")
                scan = pool.tile([128, ns], mybir.dt.float32, tag="scan")
                nc.sync.dma_start(v[:], vals_d[:])
                nc.sync.dma_start(k[:], kmask_d[:])
                nc.sync.dma_start(g[:], gidx_d[:])
                nc.vector.tensor_tensor_scan(
                    scan[:], k[:], v[:], 0.0,
                    op0=mybir.AluOpType.mult, op1=mybir.AluOpType.add,
                )
                for w in range(wr):
                    t = trims[r][w]
                    go = iop.tile([128, t], mybir.dt.float32, tag="gout")
                    nc.gpsimd.ap_gather(
                        go[:],
                        scan[:, w * SLICE : (w + 1) * SLICE],
                        g[:, (w * W) // 16 : (w * W + t) // 16],
                        channels=128, num_elems=SLICE, d=1, num_idxs=t,
                    )
                    nc.sync.dma_start(outs[r][:, w * W : w * W + t], go[:])
    nc.compile()
    return nc


_CACHE = {}


def kernel(polar_frames, flat_voxel_indices):
    polar = np.asarray(polar_frames, dtype=np.float32).reshape(B, N_CELLS)
    idx_key = np.asarray(flat_voxel_indices).tobytes()[:256]
    if idx_key in _CACHE:
        plan, nc = _CACHE[idx_key]
    else:
        plan = _build_plan(flat_voxel_indices)
        nc = _build_nc(plan["R"], plan["WR"], plan["trims"])
        _CACHE[idx_key] = (plan, nc)

    R, WR = plan["R"], plan["WR"]
    in_maps = []
    for c in range(N_CORES):
        m = {}
        for r in range(R):
            pc = plan["per_core"][c][r]
            ns = WR[r] * SLICE
            cell_src = pc["cell_src"]  # [8, ns], -1 = pad
            vals = np.zeros((N_SLOTS, B, ns), dtype=np.float32)
            valid = cell_src >= 0
            for s in range(N_SLOTS):
                vs = valid[s]
                vals[s, :, vs] = polar[:, cell_src[s, vs]].T
            m[f"vals{r}"] = vals.reshape(128, ns)
            m[f"kmask{r}"] = np.repeat(pc["kmask"], B, axis=0).reshape(128, ns).astype(
                np.uint8
            )
            gw = np.zeros((N_SLOTS, 16, (WR[r] * W) // 16), dtype=np.int16)
            for s in range(N_SLOTS):
                gw[s] = _wrap_idx(pc["gidx"][s])
            m[f"gidx{r}"] = gw.reshape(128, (WR[r] * W) // 16)
        in_maps.append(m)

    res = run_bass_kernel_spmd(nc, in_maps, core_ids=list(range(N_CORES)))

    out = np.zeros((B, N_VOX), dtype=np.float32)
    for c in range(N_CORES):
        for r in range(R):
            dense = res.results[c][f"dense{r}"].reshape(N_SLOTS, B, WR[r] * W)
            pc = plan["per_core"][c][r]
            for s in range(N_SLOTS):
                for (win, vlo, ln) in pc["spans"][s]:
                    out[:, vlo : vlo + ln] = dense[s, :, win * W : win * W + ln]
    return out.reshape(B, 1, GRID_Z, GRID_Y, GRID_X)


# revision 9
# speedup vs baseline: 3.7961x; 1.1405x over previous
"""PolarToCartesianGrid scatter-add kernel for 8 Trainium2 NeuronCores.

Strategy (voxel-range sharded, all 16 batch samples as partition lanes):
  host: sort polar cells by target voxel (indices are compile-time data);
        cut the sorted stream into 2048-cell window-slices (each covering a
        <=4096-voxel span of segment ends); oversized segments become chains
        of full "feeder" slices whose running sum continues into the next
        slice; LPT-pack chains into 64 slot-streams (8 cores x 8 groups).
  device (per core, per round):
        - DMA in values [128, WR*2048] fp32 (partition = 16*slot + lane)
        - tensor_tensor_scan(mult,add) with a bf16 reset mask => running
          segment sums; each voxel's total sits at its segment-end position
        - per window w: ap_gather from ONLY the 2048-cell slice w (small
          source window => ~3.5us/window instead of 23us) expanding segment
          ends to a dense 4096-voxel layout; untouched voxels read a
          guaranteed-zero pad slot of the slice
        - DMA each dense window out contiguously
  host: place each window's exact voxel span into the zero output buffer.
"""

import numpy as np

from concourse import bacc, mybir, tile
from concourse.bass_utils import run_bass_kernel_spmd

B = 16
N_CELLS = 1048576
GRID_X, GRID_Y, GRID_Z = 320, 320, 80
N_VOX = GRID_X * GRID_Y * GRID_Z
N_CORES = 8
N_SLOTS = 8          # partition groups per core (16 lanes each)
SLICE = 2048         # stream cells per window-slice
SLICE_CAP = 2047     # normal slices reserve >=1 zero pad
W = 4096             # dense voxels out per window
WPR = 4              # max windows per slot per round (SBUF bound)


class _Slice:
    __slots__ = ("cells", "kvec", "ends", "zslot", "used")

    def __init__(self):
        self.cells = np.full(SLICE, -1, dtype=np.int64)
        self.kvec = np.zeros(SLICE, dtype=np.float32)  # pads: K=0 (reset), v=0
        self.ends = []  # (local_pos, voxel)
        self.zslot = 0
        self.used = 0


def _build_plan(flat_idx):
    v = np.asarray(flat_idx, dtype=np.int64)
    order = np.argsort(v, kind="stable")
    sv = v[order]

    change = np.empty(N_CELLS, dtype=bool)
    change[0] = True
    change[1:] = sv[1:] != sv[:-1]
    seg_starts = np.flatnonzero(change)
    n_seg = seg_starts.size
    seg_vox = sv[seg_starts]
    seg_cnt = np.empty(n_seg, dtype=np.int64)
    seg_cnt[:-1] = np.diff(seg_starts)
    seg_cnt[-1] = N_CELLS - seg_starts[-1]

    chains = []          # list of lists of _Slice
    cur_chain = [None]   # box for current chain under construction

    def new_slice(linked):
        s = _Slice()
        if linked and cur_chain[0] is not None:
            cur_chain[0].append(s)
        else:
            cur_chain[0] = [s]
            chains.append(cur_chain[0])
        return s

    def finalize(s):
        s.zslot = s.used if s.used < SLICE else 0

    cur = new_slice(False)
    for i in range(n_seg):
        cnt = int(seg_cnt[i])
        vox = int(seg_vox[i])
        ca = int(seg_starts[i])
        if cnt <= SLICE_CAP:
            fits_cells = cur.used + cnt <= SLICE_CAP
            fits_span = (not cur.ends) or (vox - cur.ends[0][1] < W)
            if not (fits_cells and fits_span):
                finalize(cur)
                cur = new_slice(False)
            p = cur.used
            cur.cells[p : p + cnt] = order[ca : ca + cnt]
            cur.kvec[p] = 0.0
            cur.kvec[p + 1 : p + cnt] = 1.0
            cur.ends.append((p + cnt - 1, vox))
            cur.used += cnt
        else:
            # oversized segment: close current, emit full feeder slices whose
            # running sum continues into the chain's tail slice
            if cur.used > 0 or cur.ends:
                finalize(cur)
                cur = new_slice(False)
            remaining = cnt
            off = ca
            first = True
            while remaining > SLICE_CAP:
                take = min(SLICE, remaining - 1)  # keep >=1 cell for the tail
                f = cur if first else new_slice(True)
                f.cells[:take] = order[off : off + take]
                f.kvec[:] = 1.0  # trailing pads: K=1, v=0 (continue, add 0)
                if first:
                    f.kvec[0] = 0.0
                f.used = SLICE
                f.zslot = 0  # feeder: no outputs gathered
                cur = f
                remaining -= take
                off += take
                first = False
            tail = new_slice(True)
            tail.cells[:remaining] = order[off : off + remaining]
            tail.kvec[0] = 1.0  # continues the feeder chain's running sum
            tail.kvec[1:remaining] = 1.0
            tail.ends.append((remaining - 1, vox))
            tail.used = remaining
            cur = tail
    finalize(cur)

    for ch in chains:
        assert len(ch) <= WPR, "segment chain exceeds one round's window budget"

    # LPT: assign chains to 64 slots balancing window counts, then split each
    # slot's chain list into rounds of <= WPR windows
    slot_loads = [0] * (N_CORES * N_SLOTS)
    slot_chains = [[] for _ in range(N_CORES * N_SLOTS)]
    for ch in sorted(chains, key=len, reverse=True):
        s = int(np.argmin(slot_loads))
        slot_chains[s].append(ch)
        slot_loads[s] += len(ch)

    slot_rounds = []  # [slot] -> list of rounds, each a list of _Slice
    for s in range(N_CORES * N_SLOTS):
        rounds = [[]]
        for ch in slot_chains[s]:
            if len(rounds[-1]) + len(ch) > WPR:
                rounds.append([])
            rounds[-1].extend(ch)
        slot_rounds.append(rounds)

    R = max(len(r) for r in slot_rounds)
    WR = []
    for r in range(R):
        WR.append(
            max(
                (len(sr[r]) if r < len(sr) else 0)
                for sr in slot_rounds
            )
        )
    WR = [max(w, 1) for w in WR]

    plan = {"R": R, "WR": WR, "per_core": [], "trims": []}
    empty = _Slice()
    finalize(empty)
    for c in range(N_CORES):
        core_rounds = []
        for r in range(R):
            wr = WR[r]
            cell_src = np.full((N_SLOTS, wr * SLICE), -1, dtype=np.int64)
            kmask = np.zeros((N_SLOTS, wr * SLICE), dtype=np.float32)
            gidx = np.zeros((N_SLOTS, wr * W), dtype=np.int32)
            spans = [[] for _ in range(N_SLOTS)]  # (win, vox_lo, len)
            for s in range(N_SLOTS):
                sr = slot_rounds[c * N_SLOTS + s]
                wins = sr[r] if r < len(sr) else []
                for w in range(wr):
                    sl = wins[w] if w < len(wins) else empty
                    cell_src[s, w * SLICE : (w + 1) * SLICE] = sl.cells
                    kmask[s, w * SLICE : (w + 1) * SLICE] = sl.kvec
                    gidx[s, w * W : (w + 1) * W] = sl.zslot
                    if sl.ends:
                        lo = sl.ends[0][1]
                        hi = sl.ends[-1][1] + 1
                        for (pos, vox) in sl.ends:
                            gidx[s, w * W + (vox - lo)] = pos
                        spans[s].append((w, lo, hi - lo))
            core_rounds.append(
                {"cell_src": cell_src, "kmask": kmask, "gidx": gidx, "spans": spans}
            )
        plan["per_core"].append(core_rounds)
    # per (round, window): max span across all cores/slots, rounded up to 16
    trims = []
    for r in range(R):
        tr = []
        for w in range(WR[r]):
            mx = 16
            for c in range(N_CORES):
                for sp in plan["per_core"][c][r]["spans"]:
                    for (win, _lo, ln) in sp:
                        if win == w:
                            mx = max(mx, ln)
            tr.append(min(W, -(-mx // 16) * 16))
        trims.append(tr)
    plan["trims"] = trims
    return plan


def _wrap_idx(gidx_slot):
    """[NI] int -> wrapped [16, NI//16] int16 (idx j -> partition j%16, col j//16)."""
    ni = gidx_slot.size
    return np.ascontiguousarray(gidx_slot.astype(np.int16).reshape(ni // 16, 16).T)


def _build_nc(R, WR, trims):
    nc = bacc.Bacc("TRN2", target_bir_lowering=False)
    ins, outs = [], []
    for r in range(R):
        ns = WR[r] * SLICE
        ins.append(
            (
                nc.dram_tensor(f"vals{r}", (128, ns), mybir.dt.float32, kind="ExternalInput"),
                nc.dram_tensor(f"kmask{r}", (128, ns), mybir.dt.uint8, kind="ExternalInput"),
                nc.dram_tensor(f"gidx{r}", (128, (WR[r] * W) // 16), mybir.dt.int16, kind="ExternalInput"),
            )
        )
        outs.append(
            nc.dram_tensor(f"dense{r}", (128, WR[r] * W), mybir.dt.float32, kind="ExternalOutput")
        )
    with tile.TileContext(nc) as tc:
        with tc.tile_pool(name="sbuf", bufs=2) as pool, tc.tile_pool(name="io", bufs=2) as iop:
            for r in range(R):
                vals_d, kmask_d, gidx_d = ins[r]
                wr = WR[r]
                ns = wr * SLICE
                v = pool.tile([128, ns], mybir.dt.float32, tag="vals")
                k = pool.tile([128, ns], mybir.dt.uint8, tag="kmask")
                g = pool.tile([128, (wr * W) // 16], mybir.dt.int16, tag="gidx")
                scan = pool.tile([128, ns], mybir.dt.float32, tag="scan")
                nc.sync.dma_start(g[:], gidx_d[:])
                for w in range(wr):
                    sl = slice(w * SLICE, (w + 1) * SLICE)
                    nc.sync.dma_start(v[:, sl], vals_d[:, sl])
                    nc.sync.dma_start(k[:, sl], kmask_d[:, sl])
                    # chained per-window scan: state carries across windows so
                    # feeder chains keep accumulating; gathers start per window
                    nc.vector.tensor_tensor_scan(
                        scan[:, sl], k[:, sl], v[:, sl],
                        0.0 if w == 0 else scan[:, w * SLICE - 1 : w * SLICE],
                        op0=mybir.AluOpType.mult, op1=mybir.AluOpType.add,
                    )
                for w in range(wr):
                    t = trims[r][w]
                    go = iop.tile([128, t], mybir.dt.float32, tag="gout")
                    nc.gpsimd.ap_gather(
                        go[:],
                        scan[:, w * SLICE : (w + 1) * SLICE],
                        g[:, (w * W) // 16 : (w * W + t) // 16],
                        channels=128, num_elems=SLICE, d=1, num_idxs=t,
                    )
                    nc.sync.dma_start(outs[r][:, w * W : w * W + t], go[:])
    nc.compile()
    return nc


_CACHE = {}


def kernel(polar_frames, flat_voxel_indices):
    polar = np.asarray(polar_frames, dtype=np.float32).reshape(B, N_CELLS)
    idx_key = np.asarray(flat_voxel_indices).tobytes()[:256]
    if idx_key in _CACHE:
        plan, nc = _CACHE[idx_key]
    else:
        plan = _build_plan(flat_voxel_indices)
        nc = _build_nc(plan["R"], plan["WR"], plan["trims"])
        _CACHE[idx_key] = (plan, nc)

    R, WR = plan["R"], plan["WR"]
    in_maps = []
    for c in range(N_CORES):
        m = {}
        for r in range(R):
            pc = plan["per_core"][c][r]
            ns = WR[r] * SLICE
            cell_src = pc["cell_src"]  # [8, ns], -1 = pad
            vals = np.zeros((N_SLOTS, B, ns), dtype=np.float32)
            valid = cell_src >= 0
            for s in range(N_SLOTS):
                vs = valid[s]
                vals[s, :, vs] = polar[:, cell_src[s, vs]].T
            m[f"vals{r}"] = vals.reshape(128, ns)
            m[f"kmask{r}"] = np.repeat(pc["kmask"], B, axis=0).reshape(128, ns).astype(
                np.uint8
            )
            gw = np.zeros((N_SLOTS, 16, (WR[r] * W) // 16), dtype=np.int16)
            for s in range(N_SLOTS):
                gw[s] = _wrap_idx(pc["gidx"][s])
            m[f"gidx{r}"] = gw.reshape(128, (WR[r] * W) // 16)
        in_maps.append(m)

    res = run_bass_kernel_spmd(nc, in_maps, core_ids=list(range(N_CORES)))

    out = np.zeros((B, N_VOX), dtype=np.float32)
    for c in range(N_CORES):
        for r in range(R):
            dense = res.results[c][f"dense{r}"].reshape(N_SLOTS, B, WR[r] * W)
            pc = plan["per_core"][c][r]
            for s in range(N_SLOTS):
                for (win, vlo, ln) in pc["spans"][s]:
                    out[:, vlo : vlo + ln] = dense[s, :, win * W : win * W + ln]
    return out.reshape(B, 1, GRID_Z, GRID_Y, GRID_X)


# revision 10
# speedup vs baseline: 3.8394x; 1.0114x over previous
"""PolarToCartesianGrid scatter-add kernel for 8 Trainium2 NeuronCores.

Strategy (voxel-range sharded, all 16 batch samples as partition lanes):
  host: sort polar cells by target voxel (indices are compile-time data);
        cut the sorted stream into 2048-cell window-slices (each covering a
        <=4096-voxel span of segment ends); oversized segments become chains
        of full "feeder" slices whose running sum continues into the next
        slice; LPT-pack chains into 64 slot-streams (8 cores x 8 groups).
  device (per core, per round):
        - DMA in values [128, WR*2048] fp32 (partition = 16*slot + lane)
        - tensor_tensor_scan(mult,add) with a bf16 reset mask => running
          segment sums; each voxel's total sits at its segment-end position
        - per window w: ap_gather from ONLY the 2048-cell slice w (small
          source window => ~3.5us/window instead of 23us) expanding segment
          ends to a dense 4096-voxel layout; untouched voxels read a
          guaranteed-zero pad slot of the slice
        - DMA each dense window out contiguously
  host: place each window's exact voxel span into the zero output buffer.
"""

import numpy as np

from concourse import bacc, mybir, tile
from concourse.bass_utils import run_bass_kernel_spmd

B = 16
N_CELLS = 1048576
GRID_X, GRID_Y, GRID_Z = 320, 320, 80
N_VOX = GRID_X * GRID_Y * GRID_Z
N_CORES = 8
N_SLOTS = 8          # partition groups per core (16 lanes each)
SLICE = 2048         # stream cells per window-slice
SLICE_CAP = 2047     # normal slices reserve >=1 zero pad
W = 4096             # dense voxels out per window
WPR = 4              # max windows per slot per round (SBUF bound)


class _Slice:
    __slots__ = ("cells", "kvec", "ends", "zslot", "used")

    def __init__(self):
        self.cells = np.full(SLICE, -1, dtype=np.int64)
        self.kvec = np.zeros(SLICE, dtype=np.float32)  # pads: K=0 (reset), v=0
        self.ends = []  # (local_pos, voxel)
        self.zslot = 0
        self.used = 0


def _build_plan(flat_idx):
    v = np.asarray(flat_idx, dtype=np.int64)
    order = np.argsort(v, kind="stable")
    sv = v[order]

    change = np.empty(N_CELLS, dtype=bool)
    change[0] = True
    change[1:] = sv[1:] != sv[:-1]
    seg_starts = np.flatnonzero(change)
    n_seg = seg_starts.size
    seg_vox = sv[seg_starts]
    seg_cnt = np.empty(n_seg, dtype=np.int64)
    seg_cnt[:-1] = np.diff(seg_starts)
    seg_cnt[-1] = N_CELLS - seg_starts[-1]

    chains = []          # list of lists of _Slice
    cur_chain = [None]   # box for current chain under construction

    def new_slice(linked):
        s = _Slice()
        if linked and cur_chain[0] is not None:
            cur_chain[0].append(s)
        else:
            cur_chain[0] = [s]
            chains.append(cur_chain[0])
        return s

    def finalize(s):
        s.zslot = s.used if s.used < SLICE else 0

    cur = new_slice(False)
    for i in range(n_seg):
        cnt = int(seg_cnt[i])
        vox = int(seg_vox[i])
        ca = int(seg_starts[i])
        if cnt <= SLICE_CAP:
            fits_cells = cur.used + cnt <= SLICE_CAP
            fits_span = (not cur.ends) or (vox - cur.ends[0][1] < W)
            if not (fits_cells and fits_span):
                finalize(cur)
                cur = new_slice(False)
            p = cur.used
            cur.cells[p : p + cnt] = order[ca : ca + cnt]
            cur.kvec[p] = 0.0
            cur.kvec[p + 1 : p + cnt] = 1.0
            cur.ends.append((p + cnt - 1, vox))
            cur.used += cnt
        else:
            # oversized segment: close current, emit full feeder slices whose
            # running sum continues into the chain's tail slice
            if cur.used > 0 or cur.ends:
                finalize(cur)
                cur = new_slice(False)
            remaining = cnt
            off = ca
            first = True
            while remaining > SLICE_CAP:
                take = min(SLICE, remaining - 1)  # keep >=1 cell for the tail
                f = cur if first else new_slice(True)
                f.cells[:take] = order[off : off + take]
                f.kvec[:] = 1.0  # trailing pads: K=1, v=0 (continue, add 0)
                if first:
                    f.kvec[0] = 0.0
                f.used = SLICE
                f.zslot = 0  # feeder: no outputs gathered
                cur = f
                remaining -= take
                off += take
                first = False
            tail = new_slice(True)
            tail.cells[:remaining] = order[off : off + remaining]
            tail.kvec[0] = 1.0  # continues the feeder chain's running sum
            tail.kvec[1:remaining] = 1.0
            tail.ends.append((remaining - 1, vox))
            tail.used = remaining
            cur = tail
    finalize(cur)

    for ch in chains:
        assert len(ch) <= WPR, "segment chain exceeds one round's window budget"

    # LPT: assign chains to 64 slots balancing window counts, then split each
    # slot's chain list into rounds of <= WPR windows
    slot_loads = [0] * (N_CORES * N_SLOTS)
    slot_chains = [[] for _ in range(N_CORES * N_SLOTS)]
    for ch in sorted(chains, key=len, reverse=True):
        s = int(np.argmin(slot_loads))
        slot_chains[s].append(ch)
        slot_loads[s] += len(ch)

    slot_rounds = []  # [slot] -> list of rounds, each a list of _Slice
    for s in range(N_CORES * N_SLOTS):
        rounds = [[]]
        for ch in slot_chains[s]:
            if len(rounds[-1]) + len(ch) > WPR:
                rounds.append([])
            rounds[-1].extend(ch)
        slot_rounds.append(rounds)

    R = max(len(r) for r in slot_rounds)
    WR = []
    for r in range(R):
        WR.append(
            max(
                (len(sr[r]) if r < len(sr) else 0)
                for sr in slot_rounds
            )
        )
    WR = [max(w, 1) for w in WR]

    # Reorder each slot-round: keep chain blocks intact (they were packed
    # consecutively), order blocks by max used desc so window lengths align
    # tightly across slots; then compute per-window stream lengths.
    slice_chain = {}
    for ch in chains:
        for s_ in ch:
            slice_chain[id(s_)] = id(ch[0])
    for srs in slot_rounds:
        for ri in range(len(srs)):
            blocks, cur_b, cur_key = [], [], None
            for s_ in srs[ri]:
                key = slice_chain[id(s_)]
                if key != cur_key and cur_b:
                    blocks.append(cur_b)
                    cur_b = []
                cur_b.append(s_)
                cur_key = key
            if cur_b:
                blocks.append(cur_b)
            blocks.sort(key=lambda b: max(x.used for x in b), reverse=True)
            srs[ri] = [x for b in blocks for x in b]

    lens, offs = [], []
    for r in range(R):
        lw = []
        for w in range(WR[r]):
            mx = 16
            for sr in slot_rounds:
                if r < len(sr) and w < len(sr[r]):
                    sl = sr[r][w]
                    mx = max(mx, sl.used + (1 if sl.ends else 0))
            lw.append(min(SLICE, -(-mx // 16) * 16))
        ow = np.concatenate([[0], np.cumsum(lw)]).astype(np.int64)
        lens.append(lw)
        offs.append(ow)

    plan = {"R": R, "WR": WR, "per_core": [], "trims": [],
            "lens": lens, "offs": offs}
    empty = _Slice()
    finalize(empty)
    for c in range(N_CORES):
        core_rounds = []
        for r in range(R):
            wr = WR[r]
            ns = int(offs[r][wr])
            cell_src = np.full((N_SLOTS, ns), -1, dtype=np.int64)
            kmask = np.zeros((N_SLOTS, ns), dtype=np.float32)
            gidx = np.zeros((N_SLOTS, wr * W), dtype=np.int32)
            spans = [[] for _ in range(N_SLOTS)]  # (win, vox_lo, len)
            for s in range(N_SLOTS):
                sr = slot_rounds[c * N_SLOTS + s]
                wins = sr[r] if r < len(sr) else []
                for w in range(wr):
                    sl = wins[w] if w < len(wins) else empty
                    o, lw = int(offs[r][w]), lens[r][w]
                    cell_src[s, o : o + lw] = sl.cells[:lw]
                    kmask[s, o : o + lw] = sl.kvec[:lw]
                    gidx[s, w * W : (w + 1) * W] = sl.zslot
                    if sl.ends:
                        lo = sl.ends[0][1]
                        hi = sl.ends[-1][1] + 1
                        for (pos, vox) in sl.ends:
                            gidx[s, w * W + (vox - lo)] = pos
                        spans[s].append((w, lo, hi - lo))
            core_rounds.append(
                {"cell_src": cell_src, "kmask": kmask, "gidx": gidx, "spans": spans}
            )
        plan["per_core"].append(core_rounds)
    # per (round, window): max span across all cores/slots, rounded up to 16
    trims = []
    for r in range(R):
        tr = []
        for w in range(WR[r]):
            mx = 16
            for c in range(N_CORES):
                for sp in plan["per_core"][c][r]["spans"]:
                    for (win, _lo, ln) in sp:
                        if win == w:
                            mx = max(mx, ln)
            tr.append(min(W, -(-mx // 16) * 16))
        trims.append(tr)
    plan["trims"] = trims
    return plan


def _wrap_idx(gidx_slot):
    """[NI] int -> wrapped [16, NI//16] int16 (idx j -> partition j%16, col j//16)."""
    ni = gidx_slot.size
    return np.ascontiguousarray(gidx_slot.astype(np.int16).reshape(ni // 16, 16).T)


def _build_nc(R, WR, trims, lens, offs):
    nc = bacc.Bacc("TRN2", target_bir_lowering=False)
    ins, outs = [], []
    for r in range(R):
        ns = int(offs[r][WR[r]])
        ins.append(
            (
                nc.dram_tensor(f"vals{r}", (128, ns), mybir.dt.float32, kind="ExternalInput"),
                nc.dram_tensor(f"kmask{r}", (128, ns), mybir.dt.uint8, kind="ExternalInput"),
                nc.dram_tensor(f"gidx{r}", (128, (WR[r] * W) // 16), mybir.dt.int16, kind="ExternalInput"),
            )
        )
        outs.append(
            nc.dram_tensor(f"dense{r}", (128, WR[r] * W), mybir.dt.float32, kind="ExternalOutput")
        )
    with tile.TileContext(nc) as tc:
        with tc.tile_pool(name="sbuf", bufs=2) as pool, tc.tile_pool(name="io", bufs=2) as iop:
            for r in range(R):
                vals_d, kmask_d, gidx_d = ins[r]
                wr = WR[r]
                ns = int(offs[r][wr])
                v = pool.tile([128, ns], mybir.dt.float32, tag="vals")
                k = pool.tile([128, ns], mybir.dt.uint8, tag="kmask")
                g = pool.tile([128, (wr * W) // 16], mybir.dt.int16, tag="gidx")
                scan = pool.tile([128, ns], mybir.dt.float32, tag="scan")
                nc.sync.dma_start(g[:], gidx_d[:])
                for w in range(wr):
                    o, lw = int(offs[r][w]), lens[r][w]
                    sl = slice(o, o + lw)
                    nc.sync.dma_start(v[:, sl], vals_d[:, sl])
                    nc.sync.dma_start(k[:, sl], kmask_d[:, sl])
                    # chained per-window scan: state carries across windows so
                    # feeder chains keep accumulating; gathers start per window
                    nc.vector.tensor_tensor_scan(
                        scan[:, sl], k[:, sl], v[:, sl],
                        0.0 if w == 0 else scan[:, o - 1 : o],
                        op0=mybir.AluOpType.mult, op1=mybir.AluOpType.add,
                    )
                for w in range(wr):
                    o, lw = int(offs[r][w]), lens[r][w]
                    t = trims[r][w]
                    go = iop.tile([128, t], mybir.dt.float32, tag="gout")
                    nc.gpsimd.ap_gather(
                        go[:],
                        scan[:, o : o + lw],
                        g[:, (w * W) // 16 : (w * W + t) // 16],
                        channels=128, num_elems=lw, d=1, num_idxs=t,
                    )
                    nc.sync.dma_start(outs[r][:, w * W : w * W + t], go[:])
    nc.compile()
    return nc


_CACHE = {}


def kernel(polar_frames, flat_voxel_indices):
    polar = np.asarray(polar_frames, dtype=np.float32).reshape(B, N_CELLS)
    idx_key = np.asarray(flat_voxel_indices).tobytes()[:256]
    if idx_key in _CACHE:
        plan, nc = _CACHE[idx_key]
    else:
        plan = _build_plan(flat_voxel_indices)
        nc = _build_nc(plan["R"], plan["WR"], plan["trims"], plan["lens"], plan["offs"])
        _CACHE[idx_key] = (plan, nc)

    R, WR = plan["R"], plan["WR"]
    in_maps = []
    for c in range(N_CORES):
        m = {}
        for r in range(R):
            pc = plan["per_core"][c][r]
            ns = int(plan["offs"][r][WR[r]])
            cell_src = pc["cell_src"]  # [8, ns], -1 = pad
            vals = np.zeros((N_SLOTS, B, ns), dtype=np.float32)
            valid = cell_src >= 0
            for s in range(N_SLOTS):
                vs = valid[s]
                vals[s, :, vs] = polar[:, cell_src[s, vs]].T
            m[f"vals{r}"] = vals.reshape(128, ns)
            m[f"kmask{r}"] = np.repeat(pc["kmask"], B, axis=0).reshape(128, ns).astype(
                np.uint8
            )
            gw = np.zeros((N_SLOTS, 16, (WR[r] * W) // 16), dtype=np.int16)
            for s in range(N_SLOTS):
                gw[s] = _wrap_idx(pc["gidx"][s])
            m[f"gidx{r}"] = gw.reshape(128, (WR[r] * W) // 16)
        in_maps.append(m)

    res = run_bass_kernel_spmd(nc, in_maps, core_ids=list(range(N_CORES)))

    out = np.zeros((B, N_VOX), dtype=np.float32)
    for c in range(N_CORES):
        for r in range(R):
            dense = res.results[c][f"dense{r}"].reshape(N_SLOTS, B, WR[r] * W)
            pc = plan["per_core"][c][r]
            for s in range(N_SLOTS):
                for (win, vlo, ln) in pc["spans"][s]:
                    out[:, vlo : vlo + ln] = dense[s, :, win * W : win * W + ln]
    return out.reshape(B, 1, GRID_Z, GRID_Y, GRID_X)


# revision 11
# speedup vs baseline: 4.0742x; 1.0612x over previous
"""PolarToCartesianGrid scatter-add kernel for 8 Trainium2 NeuronCores.

Strategy (voxel-range sharded, all 16 batch samples as partition lanes):
  host: sort polar cells by target voxel (indices are compile-time data);
        cut the sorted stream into 2048-cell window-slices (each covering a
        <=4096-voxel span of segment ends); oversized segments become chains
        of full "feeder" slices whose running sum continues into the next
        slice; LPT-pack chains into 64 slot-streams (8 cores x 8 groups).
  device (per core, per round):
        - DMA in values [128, WR*2048] fp32 (partition = 16*slot + lane)
        - tensor_tensor_scan(mult,add) with a bf16 reset mask => running
          segment sums; each voxel's total sits at its segment-end position
        - per window w: ap_gather from ONLY the 2048-cell slice w (small
          source window => ~3.5us/window instead of 23us) expanding segment
          ends to a dense 4096-voxel layout; untouched voxels read a
          guaranteed-zero pad slot of the slice
        - DMA each dense window out contiguously
  host: place each window's exact voxel span into the zero output buffer.
"""

import numpy as np

from concourse import bacc, mybir, tile
from concourse.bass_utils import run_bass_kernel_spmd

B = 16
N_CELLS = 1048576
GRID_X, GRID_Y, GRID_Z = 320, 320, 80
N_VOX = GRID_X * GRID_Y * GRID_Z
N_CORES = 8
N_SLOTS = 8          # partition groups per core (16 lanes each)
SLICE = 2048         # stream cells per window-slice
SLICE_CAP = 2047     # normal slices reserve >=1 zero pad
W = 4096             # dense voxels out per window
WPR = 4              # max windows per slot per round (SBUF bound)


class _Slice:
    __slots__ = ("cells", "kvec", "ends", "zslot", "used")

    def __init__(self):
        self.cells = np.full(SLICE, -1, dtype=np.int64)
        self.kvec = np.zeros(SLICE, dtype=np.float32)  # pads: K=0 (reset), v=0
        self.ends = []  # (local_pos, voxel)
        self.zslot = 0
        self.used = 0


def _build_plan(flat_idx):
    v = np.asarray(flat_idx, dtype=np.int64)
    order = np.argsort(v, kind="stable")
    sv = v[order]

    change = np.empty(N_CELLS, dtype=bool)
    change[0] = True
    change[1:] = sv[1:] != sv[:-1]
    seg_starts = np.flatnonzero(change)
    n_seg = seg_starts.size
    seg_vox = sv[seg_starts]
    seg_cnt = np.empty(n_seg, dtype=np.int64)
    seg_cnt[:-1] = np.diff(seg_starts)
    seg_cnt[-1] = N_CELLS - seg_starts[-1]

    chains = []          # list of lists of _Slice
    cur_chain = [None]   # box for current chain under construction

    def new_slice(linked):
        s = _Slice()
        if linked and cur_chain[0] is not None:
            cur_chain[0].append(s)
        else:
            cur_chain[0] = [s]
            chains.append(cur_chain[0])
        return s

    def finalize(s):
        s.zslot = s.used if s.used < SLICE else 0

    cur = new_slice(False)
    for i in range(n_seg):
        cnt = int(seg_cnt[i])
        vox = int(seg_vox[i])
        ca = int(seg_starts[i])
        if cnt <= SLICE_CAP:
            fits_cells = cur.used + cnt <= SLICE_CAP
            fits_span = (not cur.ends) or (vox - cur.ends[0][1] < W)
            if not (fits_cells and fits_span):
                finalize(cur)
                cur = new_slice(False)
            p = cur.used
            cur.cells[p : p + cnt] = order[ca : ca + cnt]
            cur.kvec[p] = 0.0
            cur.kvec[p + 1 : p + cnt] = 1.0
            cur.ends.append((p + cnt - 1, vox))
            cur.used += cnt
        else:
            # oversized segment: close current, emit full feeder slices whose
            # running sum continues into the chain's tail slice
            if cur.used > 0 or cur.ends:
                finalize(cur)
                cur = new_slice(False)
            remaining = cnt
            off = ca
            first = True
            while remaining > SLICE_CAP:
                take = min(SLICE, remaining - 1)  # keep >=1 cell for the tail
                f = cur if first else new_slice(True)
                f.cells[:take] = order[off : off + take]
                f.kvec[:] = 1.0  # trailing pads: K=1, v=0 (continue, add 0)
                if first:
                    f.kvec[0] = 0.0
                f.used = SLICE
                f.zslot = 0  # feeder: no outputs gathered
                cur = f
                remaining -= take
                off += take
                first = False
            tail = new_slice(True)
            tail.cells[:remaining] = order[off : off + remaining]
            tail.kvec[0] = 1.0  # continues the feeder chain's running sum
            tail.kvec[1:remaining] = 1.0
            tail.ends.append((remaining - 1, vox))
            tail.used = remaining
            cur = tail
    finalize(cur)

    for ch in chains:
        assert len(ch) <= WPR, "segment chain exceeds one round's window budget"

    # LPT: assign chains to 64 slots balancing window counts, then split each
    # slot's chain list into rounds of <= WPR windows
    slot_loads = [0] * (N_CORES * N_SLOTS)
    slot_chains = [[] for _ in range(N_CORES * N_SLOTS)]
    for ch in sorted(chains, key=len, reverse=True):
        s = int(np.argmin(slot_loads))
        slot_chains[s].append(ch)
        slot_loads[s] += len(ch)

    slot_rounds = []  # [slot] -> list of rounds, each a list of _Slice
    for s in range(N_CORES * N_SLOTS):
        rounds = [[]]
        for ch in slot_chains[s]:
            if len(rounds[-1]) + len(ch) > WPR:
                rounds.append([])
            rounds[-1].extend(ch)
        slot_rounds.append(rounds)

    R = max(len(r) for r in slot_rounds)
    WR = []
    for r in range(R):
        WR.append(
            max(
                (len(sr[r]) if r < len(sr) else 0)
                for sr in slot_rounds
            )
        )
    WR = [max(w, 1) for w in WR]

    # Reorder each slot-round: keep chain blocks intact (they were packed
    # consecutively), order blocks by max used desc so window lengths align
    # tightly across slots; then compute per-window stream lengths.
    slice_chain = {}
    for ch in chains:
        for s_ in ch:
            slice_chain[id(s_)] = id(ch[0])
    for srs in slot_rounds:
        for ri in range(len(srs)):
            blocks, cur_b, cur_key = [], [], None
            for s_ in srs[ri]:
                key = slice_chain[id(s_)]
                if key != cur_key and cur_b:
                    blocks.append(cur_b)
                    cur_b = []
                cur_b.append(s_)
                cur_key = key
            if cur_b:
                blocks.append(cur_b)
            blocks.sort(key=lambda b: max(x.used for x in b), reverse=True)
            srs[ri] = [x for b in blocks for x in b]

    lens, offs = [], []
    for r in range(R):
        lw = []
        for w in range(WR[r]):
            mx = 16
            for sr in slot_rounds:
                if r < len(sr) and w < len(sr[r]):
                    sl = sr[r][w]
                    mx = max(mx, sl.used + (1 if sl.ends else 0))
            lw.append(min(SLICE, -(-mx // 16) * 16))
        ow = np.concatenate([[0], np.cumsum(lw)]).astype(np.int64)
        lens.append(lw)
        offs.append(ow)

    plan = {"R": R, "WR": WR, "per_core": [], "trims": [],
            "lens": lens, "offs": offs}
    empty = _Slice()
    finalize(empty)
    for c in range(N_CORES):
        core_rounds = []
        for r in range(R):
            wr = WR[r]
            ns = int(offs[r][wr])
            cell_src = np.full((N_SLOTS, ns), -1, dtype=np.int64)
            kmask = np.zeros((N_SLOTS, ns), dtype=np.float32)
            gidx = np.zeros((N_SLOTS, wr * W), dtype=np.int32)
            spans = [[] for _ in range(N_SLOTS)]  # (win, vox_lo, len)
            for s in range(N_SLOTS):
                sr = slot_rounds[c * N_SLOTS + s]
                wins = sr[r] if r < len(sr) else []
                for w in range(wr):
                    sl = wins[w] if w < len(wins) else empty
                    o, lw = int(offs[r][w]), lens[r][w]
                    cell_src[s, o : o + lw] = sl.cells[:lw]
                    kmask[s, o : o + lw] = sl.kvec[:lw]
                    gidx[s, w * W : (w + 1) * W] = sl.zslot
                    if sl.ends:
                        lo = sl.ends[0][1]
                        hi = sl.ends[-1][1] + 1
                        for (pos, vox) in sl.ends:
                            gidx[s, w * W + (vox - lo)] = pos
                        spans[s].append((w, lo, hi - lo))
            core_rounds.append(
                {"cell_src": cell_src, "kmask": kmask, "gidx": gidx, "spans": spans}
            )
        plan["per_core"].append(core_rounds)
    # per (round, window): max span across all cores/slots, rounded up to 16
    trims = []
    for r in range(R):
        tr = []
        for w in range(WR[r]):
            mx = 16
            for c in range(N_CORES):
                for sp in plan["per_core"][c][r]["spans"]:
                    for (win, _lo, ln) in sp:
                        if win == w:
                            mx = max(mx, ln)
            tr.append(min(W, -(-mx // 16) * 16))
        trims.append(tr)
    plan["trims"] = trims
    return plan


def _wrap_idx(gidx_slot):
    """[NI] int -> wrapped [16, NI//16] int16 (idx j -> partition j%16, col j//16)."""
    ni = gidx_slot.size
    return np.ascontiguousarray(gidx_slot.astype(np.int16).reshape(ni // 16, 16).T)


def _build_nc(R, WR, trims, lens, offs):
    nc = bacc.Bacc("TRN2", target_bir_lowering=False)
    ins, outs = [], []
    for r in range(R):
        ns = int(offs[r][WR[r]])
        ins.append(
            (
                nc.dram_tensor(f"vals{r}", (128, ns), mybir.dt.float32, kind="ExternalInput"),
                nc.dram_tensor(f"kmask{r}", (128, ns), mybir.dt.uint8, kind="ExternalInput"),
                nc.dram_tensor(f"gidx{r}", (128, (WR[r] * W) // 16), mybir.dt.int16, kind="ExternalInput"),
            )
        )
        outs.append(
            nc.dram_tensor(f"dense{r}", (128, WR[r] * W), mybir.dt.float32, kind="ExternalOutput")
        )
    with tile.TileContext(nc) as tc:
        with tc.tile_pool(name="sbuf", bufs=2) as pool, tc.tile_pool(name="io", bufs=3) as iop:
            for r in range(R):
                vals_d, kmask_d, gidx_d = ins[r]
                wr = WR[r]
                ns = int(offs[r][wr])
                v = pool.tile([128, ns], mybir.dt.float32, tag="vals")
                k = pool.tile([128, ns], mybir.dt.uint8, tag="kmask")
                g = pool.tile([128, (wr * W) // 16], mybir.dt.int16, tag="gidx")
                scan = pool.tile([128, ns], mybir.dt.float32, tag="scan")
                nc.sync.dma_start(g[:], gidx_d[:])
                for w in range(wr):
                    o, lw = int(offs[r][w]), lens[r][w]
                    sl = slice(o, o + lw)
                    nc.sync.dma_start(v[:, sl], vals_d[:, sl])
                    nc.sync.dma_start(k[:, sl], kmask_d[:, sl])
                    # chained per-window scan: state carries across windows so
                    # feeder chains keep accumulating; gathers start per window
                    nc.vector.tensor_tensor_scan(
                        scan[:, sl], k[:, sl], v[:, sl],
                        0.0 if w == 0 else scan[:, o - 1 : o],
                        op0=mybir.AluOpType.mult, op1=mybir.AluOpType.add,
                    )
                for w in range(wr):
                    o, lw = int(offs[r][w]), lens[r][w]
                    t = trims[r][w]
                    go = iop.tile([128, t], mybir.dt.float32, tag="gout")
                    nc.gpsimd.ap_gather(
                        go[:],
                        scan[:, o : o + lw],
                        g[:, (w * W) // 16 : (w * W + t) // 16],
                        channels=128, num_elems=lw, d=1, num_idxs=t,
                    )
                    nc.sync.dma_start(outs[r][:, w * W : w * W + t], go[:])
    nc.compile()
    return nc


_CACHE = {}


def kernel(polar_frames, flat_voxel_indices):
    polar = np.asarray(polar_frames, dtype=np.float32).reshape(B, N_CELLS)
    idx_key = np.asarray(flat_voxel_indices).tobytes()[:256]
    if idx_key in _CACHE:
        plan, nc = _CACHE[idx_key]
    else:
        plan = _build_plan(flat_voxel_indices)
        nc = _build_nc(plan["R"], plan["WR"], plan["trims"], plan["lens"], plan["offs"])
        _CACHE[idx_key] = (plan, nc)

    R, WR = plan["R"], plan["WR"]
    in_maps = []
    for c in range(N_CORES):
        m = {}
        for r in range(R):
            pc = plan["per_core"][c][r]
            ns = int(plan["offs"][r][WR[r]])
            cell_src = pc["cell_src"]  # [8, ns], -1 = pad
            vals = np.zeros((N_SLOTS, B, ns), dtype=np.float32)
            valid = cell_src >= 0
            for s in range(N_SLOTS):
                vs = valid[s]
                vals[s, :, vs] = polar[:, cell_src[s, vs]].T
            m[f"vals{r}"] = vals.reshape(128, ns)
            m[f"kmask{r}"] = np.repeat(pc["kmask"], B, axis=0).reshape(128, ns).astype(
                np.uint8
            )
            gw = np.zeros((N_SLOTS, 16, (WR[r] * W) // 16), dtype=np.int16)
            for s in range(N_SLOTS):
                gw[s] = _wrap_idx(pc["gidx"][s])
            m[f"gidx{r}"] = gw.reshape(128, (WR[r] * W) // 16)
        in_maps.append(m)

    res = run_bass_kernel_spmd(nc, in_maps, core_ids=list(range(N_CORES)))

    out = np.zeros((B, N_VOX), dtype=np.float32)
    for c in range(N_CORES):
        for r in range(R):
            dense = res.results[c][f"dense{r}"].reshape(N_SLOTS, B, WR[r] * W)
            pc = plan["per_core"][c][r]
            for s in range(N_SLOTS):
                for (win, vlo, ln) in pc["spans"][s]:
                    out[:, vlo : vlo + ln] = dense[s, :, win * W : win * W + ln]
    return out.reshape(B, 1, GRID_Z, GRID_Y, GRID_X)


# revision 12
# speedup vs baseline: 4.1739x; 1.0245x over previous
"""PolarToCartesianGrid scatter-add kernel for 8 Trainium2 NeuronCores.

Strategy (voxel-range sharded, all 16 batch samples as partition lanes):
  host: sort polar cells by target voxel (indices are compile-time data);
        cut the sorted stream into 2048-cell window-slices (each covering a
        <=4096-voxel span of segment ends); oversized segments become chains
        of full "feeder" slices whose running sum continues into the next
        slice; LPT-pack chains into 64 slot-streams (8 cores x 8 groups).
  device (per core, per round):
        - DMA in values [128, WR*2048] fp32 (partition = 16*slot + lane)
        - tensor_tensor_scan(mult,add) with a bf16 reset mask => running
          segment sums; each voxel's total sits at its segment-end position
        - per window w: ap_gather from ONLY the 2048-cell slice w (small
          source window => ~3.5us/window instead of 23us) expanding segment
          ends to a dense 4096-voxel layout; untouched voxels read a
          guaranteed-zero pad slot of the slice
        - DMA each dense window out contiguously
  host: place each window's exact voxel span into the zero output buffer.
"""

import numpy as np

from concourse import bacc, mybir, tile
from concourse.bass_utils import run_bass_kernel_spmd

B = 16
N_CELLS = 1048576
GRID_X, GRID_Y, GRID_Z = 320, 320, 80
N_VOX = GRID_X * GRID_Y * GRID_Z
N_CORES = 8
N_SLOTS = 8          # partition groups per core (16 lanes each)
SLICE = 2048         # stream cells per window-slice
SLICE_CAP = 2047     # normal slices reserve >=1 zero pad
W = 4096             # dense voxels out per window
WPR = 4              # max windows per slot per round (SBUF bound)


class _Slice:
    __slots__ = ("cells", "kvec", "ends", "zslot", "used")

    def __init__(self):
        self.cells = np.full(SLICE, -1, dtype=np.int64)
        self.kvec = np.zeros(SLICE, dtype=np.float32)  # pads: K=0 (reset), v=0
        self.ends = []  # (local_pos, voxel)
        self.zslot = 0
        self.used = 0


def _build_plan(flat_idx):
    v = np.asarray(flat_idx, dtype=np.int64)
    order = np.argsort(v, kind="stable")
    sv = v[order]

    change = np.empty(N_CELLS, dtype=bool)
    change[0] = True
    change[1:] = sv[1:] != sv[:-1]
    seg_starts = np.flatnonzero(change)
    n_seg = seg_starts.size
    seg_vox = sv[seg_starts]
    seg_cnt = np.empty(n_seg, dtype=np.int64)
    seg_cnt[:-1] = np.diff(seg_starts)
    seg_cnt[-1] = N_CELLS - seg_starts[-1]

    chains = []          # list of lists of _Slice
    cur_chain = [None]   # box for current chain under construction

    def new_slice(linked):
        s = _Slice()
        if linked and cur_chain[0] is not None:
            cur_chain[0].append(s)
        else:
            cur_chain[0] = [s]
            chains.append(cur_chain[0])
        return s

    def finalize(s):
        s.zslot = s.used if s.used < SLICE else 0

    cur = new_slice(False)
    for i in range(n_seg):
        cnt = int(seg_cnt[i])
        vox = int(seg_vox[i])
        ca = int(seg_starts[i])
        if cnt <= SLICE_CAP:
            fits_cells = cur.used + cnt <= SLICE_CAP
            fits_span = (not cur.ends) or (vox - cur.ends[0][1] < W)
            if not (fits_cells and fits_span):
                finalize(cur)
                cur = new_slice(False)
            p = cur.used
            cur.cells[p : p + cnt] = order[ca : ca + cnt]
            cur.kvec[p] = 0.0
            cur.kvec[p + 1 : p + cnt] = 1.0
            cur.ends.append((p + cnt - 1, vox))
            cur.used += cnt
        else:
            # oversized segment: close current, emit full feeder slices whose
            # running sum continues into the chain's tail slice
            if cur.used > 0 or cur.ends:
                finalize(cur)
                cur = new_slice(False)
            remaining = cnt
            off = ca
            first = True
            while remaining > SLICE_CAP:
                take = min(SLICE, remaining - 1)  # keep >=1 cell for the tail
                f = cur if first else new_slice(True)
                f.cells[:take] = order[off : off + take]
                f.kvec[:] = 1.0  # trailing pads: K=1, v=0 (continue, add 0)
                if first:
                    f.kvec[0] = 0.0
                f.used = SLICE
                f.zslot = 0  # feeder: no outputs gathered
                cur = f
                remaining -= take
                off += take
                first = False
            tail = new_slice(True)
            tail.cells[:remaining] = order[off : off + remaining]
            tail.kvec[0] = 1.0  # continues the feeder chain's running sum
            tail.kvec[1:remaining] = 1.0
            tail.ends.append((remaining - 1, vox))
            tail.used = remaining
            cur = tail
    finalize(cur)

    for ch in chains:
        assert len(ch) <= WPR, "segment chain exceeds one round's window budget"

    # Global rank-major packing: multi-slice chains pinned to distinct slots
    # first, then singleton slices sorted by used desc are dealt across slots
    # rank by rank, so each rank's max length (= window length) stays tight.
    NSLOT = N_CORES * N_SLOTS
    multi = [ch for ch in chains if len(ch) > 1]
    singles = [ch[0] for ch in chains if len(ch) == 1]
    n_win = sum(len(ch) for ch in chains)
    R = -(-max(n_win, 1) // (NSLOT * WPR))
    R = max(R, max((len(ch) + WPR - 1) // WPR for ch in chains) if chains else 1)
    while R * NSLOT * WPR < n_win:
        R += 1
    grid = [[None] * (R * WPR) for _ in range(NSLOT)]
    for i, ch in enumerate(sorted(multi, key=len, reverse=True)):
        assert i < NSLOT, "too many multi-slice chains"
        grid[i][: len(ch)] = ch
    singles.sort(key=lambda s_: s_.used, reverse=True)
    it = iter(singles)
    done = False
    for rank in range(R * WPR):
        for s in range(NSLOT):
            if grid[s][rank] is None:
                try:
                    grid[s][rank] = next(it)
                except StopIteration:
                    done = True
                    break
        if done:
            break
    assert done or next(it, None) is None

    slot_rounds = []  # [slot] -> list of rounds, each a list of _Slice
    for s in range(NSLOT):
        rounds = []
        for r in range(R):
            rounds.append([x for x in grid[s][r * WPR : (r + 1) * WPR] if x is not None])
        slot_rounds.append(rounds)
    WR = []
    for r in range(R):
        WR.append(
            max(
                (len(sr[r]) if r < len(sr) else 0)
                for sr in slot_rounds
            )
        )
    WR = [max(w, 1) for w in WR]

    # Reorder each slot-round: keep chain blocks intact (they were packed
    # consecutively), order blocks by max used desc so window lengths align
    # tightly across slots; then compute per-window stream lengths.
    slice_chain = {}
    for ch in chains:
        for s_ in ch:
            slice_chain[id(s_)] = id(ch[0])
    for srs in slot_rounds:
        for ri in range(len(srs)):
            blocks, cur_b, cur_key = [], [], None
            for s_ in srs[ri]:
                key = slice_chain[id(s_)]
                if key != cur_key and cur_b:
                    blocks.append(cur_b)
                    cur_b = []
                cur_b.append(s_)
                cur_key = key
            if cur_b:
                blocks.append(cur_b)
            blocks.sort(key=lambda b: max(x.used for x in b), reverse=True)
            srs[ri] = [x for b in blocks for x in b]

    lens, offs = [], []
    for r in range(R):
        lw = []
        for w in range(WR[r]):
            mx = 16
            for sr in slot_rounds:
                if r < len(sr) and w < len(sr[r]):
                    sl = sr[r][w]
                    mx = max(mx, sl.used + (1 if sl.ends else 0))
            lw.append(min(SLICE, -(-mx // 16) * 16))
        ow = np.concatenate([[0], np.cumsum(lw)]).astype(np.int64)
        lens.append(lw)
        offs.append(ow)

    plan = {"R": R, "WR": WR, "per_core": [], "trims": [],
            "lens": lens, "offs": offs}
    empty = _Slice()
    finalize(empty)
    for c in range(N_CORES):
        core_rounds = []
        for r in range(R):
            wr = WR[r]
            ns = int(offs[r][wr])
            cell_src = np.full((N_SLOTS, ns), -1, dtype=np.int64)
            kmask = np.zeros((N_SLOTS, ns), dtype=np.float32)
            gidx = np.zeros((N_SLOTS, wr * W), dtype=np.int32)
            spans = [[] for _ in range(N_SLOTS)]  # (win, vox_lo, len)
            for s in range(N_SLOTS):
                sr = slot_rounds[c * N_SLOTS + s]
                wins = sr[r] if r < len(sr) else []
                for w in range(wr):
                    sl = wins[w] if w < len(wins) else empty
                    o, lw = int(offs[r][w]), lens[r][w]
                    cell_src[s, o : o + lw] = sl.cells[:lw]
                    kmask[s, o : o + lw] = sl.kvec[:lw]
                    gidx[s, w * W : (w + 1) * W] = sl.zslot
                    if sl.ends:
                        lo = sl.ends[0][1]
                        hi = sl.ends[-1][1] + 1
                        for (pos, vox) in sl.ends:
                            gidx[s, w * W + (vox - lo)] = pos
                        spans[s].append((w, lo, hi - lo))
            core_rounds.append(
                {"cell_src": cell_src, "kmask": kmask, "gidx": gidx, "spans": spans}
            )
        plan["per_core"].append(core_rounds)
    # per (round, window): max span across all cores/slots, rounded up to 16
    trims = []
    for r in range(R):
        tr = []
        for w in range(WR[r]):
            mx = 16
            for c in range(N_CORES):
                for sp in plan["per_core"][c][r]["spans"]:
                    for (win, _lo, ln) in sp:
                        if win == w:
                            mx = max(mx, ln)
            tr.append(min(W, -(-mx // 16) * 16))
        trims.append(tr)
    plan["trims"] = trims
    return plan


def _wrap_idx(gidx_slot):
    """[NI] int -> wrapped [16, NI//16] int16 (idx j -> partition j%16, col j//16)."""
    ni = gidx_slot.size
    return np.ascontiguousarray(gidx_slot.astype(np.int16).reshape(ni // 16, 16).T)


def _build_nc(R, WR, trims, lens, offs):
    nc = bacc.Bacc("TRN2", target_bir_lowering=False)
    ins, outs = [], []
    for r in range(R):
        ns = int(offs[r][WR[r]])
        ins.append(
            (
                nc.dram_tensor(f"vals{r}", (128, ns), mybir.dt.float32, kind="ExternalInput"),
                nc.dram_tensor(f"kmask{r}", (128, ns), mybir.dt.uint8, kind="ExternalInput"),
                nc.dram_tensor(f"gidx{r}", (128, (WR[r] * W) // 16), mybir.dt.int16, kind="ExternalInput"),
            )
        )
        outs.append(
            nc.dram_tensor(f"dense{r}", (128, WR[r] * W), mybir.dt.float32, kind="ExternalOutput")
        )
    with tile.TileContext(nc) as tc:
        with tc.tile_pool(name="sbuf", bufs=2) as pool, tc.tile_pool(name="io", bufs=3) as iop:
            for r in range(R):
                vals_d, kmask_d, gidx_d = ins[r]
                wr = WR[r]
                ns = int(offs[r][wr])
                v = pool.tile([128, ns], mybir.dt.float32, tag="vals")
                k = pool.tile([128, ns], mybir.dt.uint8, tag="kmask")
                g = pool.tile([128, (wr * W) // 16], mybir.dt.int16, tag="gidx")
                scan = pool.tile([128, ns], mybir.dt.float32, tag="scan")
                nc.sync.dma_start(g[:], gidx_d[:])
                for w in range(wr):
                    o, lw = int(offs[r][w]), lens[r][w]
                    sl = slice(o, o + lw)
                    nc.sync.dma_start(v[:, sl], vals_d[:, sl])
                    nc.sync.dma_start(k[:, sl], kmask_d[:, sl])
                    # chained per-window scan: state carries across windows so
                    # feeder chains keep accumulating; gathers start per window
                    nc.vector.tensor_tensor_scan(
                        scan[:, sl], k[:, sl], v[:, sl],
                        0.0 if w == 0 else scan[:, o - 1 : o],
                        op0=mybir.AluOpType.mult, op1=mybir.AluOpType.add,
                    )
                for w in range(wr):
                    o, lw = int(offs[r][w]), lens[r][w]
                    t = trims[r][w]
                    go = iop.tile([128, t], mybir.dt.float32, tag="gout")
                    nc.gpsimd.ap_gather(
                        go[:],
                        scan[:, o : o + lw],
                        g[:, (w * W) // 16 : (w * W + t) // 16],
                        channels=128, num_elems=lw, d=1, num_idxs=t,
                    )
                    nc.sync.dma_start(outs[r][:, w * W : w * W + t], go[:])
    nc.compile()
    return nc


_CACHE = {}


def kernel(polar_frames, flat_voxel_indices):
    polar = np.asarray(polar_frames, dtype=np.float32).reshape(B, N_CELLS)
    idx_key = np.asarray(flat_voxel_indices).tobytes()[:256]
    if idx_key in _CACHE:
        plan, nc = _CACHE[idx_key]
    else:
        plan = _build_plan(flat_voxel_indices)
        nc = _build_nc(plan["R"], plan["WR"], plan["trims"], plan["lens"], plan["offs"])
        _CACHE[idx_key] = (plan, nc)

    R, WR = plan["R"], plan["WR"]
    in_maps = []
    for c in range(N_CORES):
        m = {}
        for r in range(R):
            pc = plan["per_core"][c][r]
            ns = int(plan["offs"][r][WR[r]])
            cell_src = pc["cell_src"]  # [8, ns], -1 = pad
            vals = np.zeros((N_SLOTS, B, ns), dtype=np.float32)
            valid = cell_src >= 0
            for s in range(N_SLOTS):
                vs = valid[s]
                vals[s, :, vs] = polar[:, cell_src[s, vs]].T
            m[f"vals{r}"] = vals.reshape(128, ns)
            m[f"kmask{r}"] = np.repeat(pc["kmask"], B, axis=0).reshape(128, ns).astype(
                np.uint8
            )
            gw = np.zeros((N_SLOTS, 16, (WR[r] * W) // 16), dtype=np.int16)
            for s in range(N_SLOTS):
                gw[s] = _wrap_idx(pc["gidx"][s])
            m[f"gidx{r}"] = gw.reshape(128, (WR[r] * W) // 16)
        in_maps.append(m)

    res = run_bass_kernel_spmd(nc, in_maps, core_ids=list(range(N_CORES)))

    out = np.zeros((B, N_VOX), dtype=np.float32)
    for c in range(N_CORES):
        for r in range(R):
            dense = res.results[c][f"dense{r}"].reshape(N_SLOTS, B, WR[r] * W)
            pc = plan["per_core"][c][r]
            for s in range(N_SLOTS):
                for (win, vlo, ln) in pc["spans"][s]:
                    out[:, vlo : vlo + ln] = dense[s, :, win * W : win * W + ln]
    return out.reshape(B, 1, GRID_Z, GRID_Y, GRID_X)
